# revision 1
# baseline (speedup 1.0000x reference)
"""Trainium2 Bass kernel for nn_DiseaseModel_mlp (GNN message passing + MLP decoder).

Data parallel over the batch dim: 64 graphs -> 8 NeuronCores x 8 graphs.
All weights replicated. Host does layout staging only (transposes/chunking);
all math from xs/A/cell_emb to score runs on device.

Note: every matmul operand is kept at SBUF base partition 0 — operands at
base 64 hard-crash the device when bases alternate across matmuls.
"""

import numpy as np

ATOM = 34
HID = 256
LATENT = 128
CELLS = 512
B, N = 64, 128
NCORES = 8
G = B // NCORES          # graphs per core = 8

_CACHE = {}


def _build_bass(dbg=0):
    """Build the Bass program (one NeuronCore, SPMD across 8).
    dbg>0: also dump the x state after round dbg (1..3) to out "xdbg"."""
    import concourse.bass as bass
    import concourse.bacc as bacc
    import concourse.mybir as mybir
    import concourse.tile as tile

    f32 = mybir.dt.float32
    AF = mybir.ActivationFunctionType
    OP = mybir.AluOpType
    AX = mybir.AxisListType

    nc = bacc.Bacc(None)

    # ---- DRAM parameters (per-core views; names match in_map keys) ----
    d_x0t = nc.declare_dram_parameter("x0t", [G, ATOM + 1, N], f32, isOutput=False)
    d_m01t = nc.declare_dram_parameter("m01t", [G, N, N], f32, isOutput=False)
    d_cell = nc.declare_dram_parameter("cell", [G, CELLS], f32, isOutput=False)
    d_wg = nc.declare_dram_parameter("wg", [3, ATOM + 1, ATOM], f32, isOutput=False)
    d_a12 = nc.declare_dram_parameter("a12", [3, 2, ATOM], f32, isOutput=False)
    d_wt = nc.declare_dram_parameter("wt", [ATOM + 1, HID], f32, isOutput=False)
    d_wf = nc.declare_dram_parameter("wf", [2, 128, ATOM], f32, isOutput=False)
    d_wf2 = nc.declare_dram_parameter("wf2", [ATOM, LATENT], f32, isOutput=False)
    d_b2n = nc.declare_dram_parameter("b2n", [LATENT, 1], f32, isOutput=False)
    d_w1 = nc.declare_dram_parameter("w1", [5, 128, 128], f32, isOutput=False)
    d_b1 = nc.declare_dram_parameter("b1", [128, 1], f32, isOutput=False)
    d_w2 = nc.declare_dram_parameter("w2", [2, 128, 128], f32, isOutput=False)
    d_b2d = nc.declare_dram_parameter("b2d", [2, 128, 1], f32, isOutput=False)
    d_w3 = nc.declare_dram_parameter("w3", [2, 4, 128, 128], f32, isOutput=False)
    d_b3 = nc.declare_dram_parameter("b3", [4, 128, 1], f32, isOutput=False)
    d_w4 = nc.declare_dram_parameter("w4", [4, 128, 1], f32, isOutput=False)
    d_b4 = nc.declare_dram_parameter("b4", [1, 1], f32, isOutput=False)
    d_i128 = nc.declare_dram_parameter("i128", [128, 128], f32, isOutput=False)
    d_score = nc.declare_dram_parameter("score", [G, 1], f32, isOutput=True)
    _dbgshape = {4: [128, G, N], 5: [ATOM, G, N], 6: [LATENT, G],
                 7: [LATENT, G], 8: [128, 4, G], 9: [128, G],
                 10: [128, 2, G], 11: [128, 4, G], 12: [1, G],
                 13: [1, G]}.get(dbg, [ATOM, G, N])
    d_xdbg = (nc.declare_dram_parameter("xdbg", _dbgshape, f32, isOutput=True)
              if dbg else None)

    with tile.TileContext(nc) as tc:
        with (
            tc.tile_pool(name="singles", bufs=1) as singles,
            tc.tile_pool(name="work", bufs=2) as work,
            tc.tile_pool(name="ps1", bufs=1, space="PSUM") as ps1,
            tc.tile_pool(name="ps2", bufs=1, space="PSUM") as ps2,
        ):
            # ACT table warm-up: first ACT instruction is an Exp so the
            # exp_and_others table loads while input DMAs are in flight.
            warm = singles.tile([1, 1], f32, tag="warm")
            nc.vector.memset(warm, 0.0)
            nc.scalar.activation(out=warm, in_=warm, func=AF.Exp)

            # x state, feature-major: rows 0:34 features, row 34 = ones
            state = singles.tile([ATOM + 1, G, N], f32, tag="state")
            for g in range(G):
                nc.sync.dma_start(out=state[:, g, :], in_=d_x0t[g])

            # initial x kept separately for the d1 residual
            x0td = singles.tile([ATOM, G, N], f32, tag="x0td")
            for g in range(G):
                nc.sync.dma_start(out=x0td[:, g, :], in_=d_x0t[g, 0:ATOM, :])

            m01t_sb = singles.tile([N, G, N], f32, tag="m01t")
            for g in range(G):
                eng = nc.sync if g % 2 == 0 else nc.scalar
                eng.dma_start(out=m01t_sb[:, g, :], in_=d_m01t[g])

            wg_sb = singles.tile([ATOM + 1, 3, ATOM], f32, tag="wg")
            nc.scalar.dma_start(out=wg_sb, in_=d_wg.rearrange("r k d -> k r d"))

            a12b = singles.tile([128, 3, 2, ATOM], f32, tag="a12b")
            a12_bcast = bass.AP(
                tensor=d_a12[:].tensor,
                offset=d_a12[:].offset,
                ap=[[0, 128]] + list(d_a12[:].ap),
            )
            nc.scalar.dma_start(out=a12b, in_=a12_bcast)

            i128_sb = singles.tile([128, 128], f32, tag="i128")
            nc.scalar.dma_start(out=i128_sb, in_=d_i128[:])

            wt_sb = singles.tile([ATOM + 1, 2, 128], f32, tag="wt")
            nc.sync.dma_start(out=wt_sb, in_=d_wt.rearrange("k (h m) -> k h m", h=2))

            wf_sb = singles.tile([128, 2, ATOM], f32, tag="wf")
            nc.sync.dma_start(out=wf_sb, in_=d_wf.rearrange("c k d -> k c d"))

            wf2_sb = singles.tile([ATOM, LATENT], f32, tag="wf2")
            nc.sync.dma_start(out=wf2_sb, in_=d_wf2[:])

            b2n_sb = singles.tile([LATENT, 1], f32, tag="b2n")
            nc.gpsimd.dma_start(out=b2n_sb, in_=d_b2n[:])

            cl_sb = singles.tile([G, CELLS], f32, tag="cell")
            nc.gpsimd.dma_start(out=cl_sb, in_=d_cell[:])

            w1_sb = singles.tile([128, 5, 128], f32, tag="w1")
            nc.gpsimd.dma_start(out=w1_sb, in_=d_w1.rearrange("c k m -> k c m"))
            b1_sb = singles.tile([128, 1], f32, tag="b1")
            nc.gpsimd.dma_start(out=b1_sb, in_=d_b1[:])
            w2_sb = singles.tile([128, 2, 128], f32, tag="w2")
            nc.gpsimd.dma_start(out=w2_sb, in_=d_w2.rearrange("b k m -> k b m"))
            b2d_sb = singles.tile([128, 2], f32, tag="b2d")
            nc.gpsimd.dma_start(out=b2d_sb, in_=d_b2d.rearrange("b k x -> k (b x)"))
            w3_sb = singles.tile([128, 2, 4, 128], f32, tag="w3")
            nc.gpsimd.dma_start(out=w3_sb, in_=d_w3.rearrange("c b k m -> k c b m"))
            b3_sb = singles.tile([128, 4], f32, tag="b3")
            nc.gpsimd.dma_start(out=b3_sb, in_=d_b3.rearrange("b k x -> k (b x)"))
            w4_sb = singles.tile([128, 4], f32, tag="w4")
            nc.gpsimd.dma_start(out=w4_sb, in_=d_w4.rearrange("c k x -> k (c x)"))
            b4_sb = singles.tile([1, 1], f32, tag="b4")
            nc.gpsimd.dma_start(out=b4_sb, in_=d_b4[:])

            ones1 = singles.tile([1, 128], f32, tag="ones1")
            nc.vector.memset(ones1, 1.0)

            # h with a trailing ones column (gives att row-sums for free)
            haug = singles.tile([N, G, ATOM + 1], f32, tag="haug")
            nc.vector.memset(haug[:, :, ATOM], 1.0)

            # ---- GNN rounds ----
            for r in range(3):
                # h = relu(x @ Wg[r] + bg[r]) in node-major layout
                h_ps = ps1.tile([N, G, ATOM], f32, tag="h_ps")
                for g in range(G):
                    nc.tensor.matmul(h_ps[:, g, :], state[:, g, :],
                                     wg_sb[:, r, :], start=True, stop=True)
                nc.vector.tensor_scalar_max(haug[:, :, 0:ATOM], h_ps, 0.0)

                # f_src/f_dst = h @ a1, h @ a2: multiply on GPSIMD, reduce on DVE
                tf = work.tile([N, 2, G, ATOM], f32, tag="tf")
                h_b = haug[:, :, 0:ATOM].unsqueeze(1).to_broadcast([N, 2, G, ATOM])
                a_b = a12b[:, r, :, :].unsqueeze(2).to_broadcast([128, 2, G, ATOM])
                nc.vector.tensor_tensor(tf, h_b, a_b, OP.mult)
                fqg = work.tile([N, 2, G], f32, tag="fqg")
                nc.vector.tensor_reduce(fqg, tf, AX.X, OP.add)

                # f_src crosses partition->free: 8 column transposes land all
                # rows on partition 0, then a rank-1 matmul fans out to 128.
                ft_ps = ps1.tile([1, G, N], f32, tag="ft_ps")
                for g in range(G):
                    nc.tensor.transpose(ft_ps[0:1, g, :], fqg[:, 0, g:g + 1],
                                        i128_sb)
                fcat = work.tile([1, G, N], f32, tag="ftsrc")
                nc.vector.tensor_copy(fcat, ft_ps)
                e_ps = ps2.tile([N, G, N], f32, tag="gt_ps")
                for hh in range(2):
                    nc.tensor.matmul(e_ps[:, 4 * hh:4 * (hh + 1), :], ones1,
                                     fcat[:, 4 * hh:4 * (hh + 1), :],
                                     start=True, stop=True)

                # e[q, (g,p)] = f_src_g[p] + f_dst_g[q]
                e_sb = work.tile([N, G, N], f32, tag="e_sb")
                fd_b = fqg[:, 1, :].unsqueeze(2).to_broadcast([N, G, N])
                nc.vector.tensor_tensor(e_sb, e_ps, fd_b, OP.add)

                # lrelu(e) = 0.01*e + relu(0.99*e); Relu/Exp share the
                # exp_and_others ACT table set, so no per-round table loads.
                r_sb = work.tile([N, G, N], f32, tag="r_sb")
                nc.scalar.activation(out=r_sb, in_=e_sb, func=AF.Relu, scale=0.99)
                t_sb = work.tile([N, G, N], f32, tag="t_sb")
                nc.vector.tensor_scalar_mul(t_sb, e_sb, 0.01)
                nc.vector.tensor_tensor(t_sb, t_sb, r_sb, OP.add)
                p_sb = work.tile([N, G, N], f32, tag="p_sb")
                nc.scalar.activation(out=p_sb, in_=t_sb, func=AF.Exp)

                # mask multiply (split DVE / GPSIMD)
                pm = work.tile([N, G, N], f32, tag="pm")
                nc.vector.tensor_tensor(pm[:, 0:4, :], p_sb[:, 0:4, :],
                                        m01t_sb[:, 0:4, :], OP.mult)
                nc.gpsimd.tensor_tensor(pm[:, 4:G, :], p_sb[:, 4:G, :],
                                        m01t_sb[:, 4:G, :], OP.mult)

                # U = P @ [h | 1]  (last col = row-sum of P)
                u_ps = ps1.tile([N, G, ATOM + 1], f32, tag="u_ps")
                for g in range(G):
                    nc.tensor.matmul(u_ps[:, g, :], pm[:, g, :], haug[:, g, :],
                                     start=True, stop=True)

                irs = work.tile([N, G], f32, tag="irs")
                nc.vector.reciprocal(irs, u_ps[:, :, ATOM])

                # delta = U * (1/rowsum) in node-major layout
                dlt = work.tile([N, G, ATOM], f32, tag="dlt")
                i_b = irs.unsqueeze(2).to_broadcast([N, G, ATOM])
                nc.vector.tensor_tensor(dlt, u_ps[:, :, 0:ATOM], i_b, OP.mult)

                # transpose deltas per graph and accumulate into the state
                dt_ps = ps1.tile([ATOM, G, N], f32, tag="dt_ps")
                for g in range(G):
                    nc.tensor.transpose(dt_ps[:, g, :], dlt[:, g, :], i128_sb)
                nc.vector.tensor_tensor(state[0:ATOM], state[0:ATOM], dt_ps,
                                        OP.add)
                if dbg == r + 1:
                    xd = work.tile([ATOM, G, N], f32, tag="xd")
                    nc.vector.tensor_copy(xd, state[0:ATOM])
                    nc.sync.dma_start(out=d_xdbg[:], in_=xd)

            # ---- g = relu(x3 @ Wt + bt), kept transposed in two 128-halves ----
            gt_sb = []
            for hh in range(2):
                gt_ps = ps2.tile([128, G, N], f32, tag="gt_ps")
                for half in range(2):
                    sl = slice(4 * half, 4 * (half + 1))
                    nc.tensor.matmul(gt_ps[:, sl, :], wt_sb[:, hh, :],
                                     state[:, sl, :], start=True, stop=True)
                gts = singles.tile([128, G, N], f32, tag=f"gt{hh}")
                nc.scalar.activation(out=gts, in_=gt_ps, func=AF.Relu)
                gt_sb.append(gts)
            if dbg == 4:
                nc.sync.dma_start(out=d_xdbg[:], in_=gt_sb[0])

            # ---- d1 = g @ Wf (+ x0 residual after transpose; bf folded in b2n)
            d1_ps = ps1.tile([N, G, ATOM], f32, tag="h_ps")
            for g in range(G):
                nc.tensor.matmul(d1_ps[:, g, :], gt_sb[0][:, g, :], wf_sb[:, 0, :],
                                 start=True, stop=False)
                nc.tensor.matmul(d1_ps[:, g, :], gt_sb[1][:, g, :], wf_sb[:, 1, :],
                                 start=False, stop=True)
            d1n = work.tile([N, G, ATOM], f32, tag="d1n")
            nc.vector.tensor_copy(d1n, d1_ps)

            d1t_ps = ps1.tile([ATOM, G, N], f32, tag="dt_ps")
            for g in range(G):
                nc.tensor.transpose(d1t_ps[:, g, :], d1n[:, g, :], i128_sb)
            d1t_sb = work.tile([ATOM, G, N], f32, tag="d1t_sb")
            nc.vector.tensor_tensor(d1t_sb, d1t_ps, x0td, OP.add)
            if dbg == 5:
                nc.sync.dma_start(out=d_xdbg[:], in_=d1t_sb)

            # ---- d2 = d1 @ Wf2 (transposed out), then max over nodes ----
            d2_ps = ps2.tile([LATENT, G, N], f32, tag="gt_ps")
            for half in range(2):
                sl = slice(4 * half, 4 * (half + 1))
                nc.tensor.matmul(d2_ps[:, sl, :], wf2_sb, d1t_sb[:, sl, :],
                                 start=True, stop=True)
            dm = work.tile([LATENT, G], f32, tag="dm")
            nc.vector.tensor_reduce(dm, d2_ps, AX.X, OP.max)
            if dbg == 6:
                nc.sync.dma_start(out=d_xdbg[:], in_=dm)

            # ---- vec = sigmoid([dmax + b2', cell]) via exp + reciprocal ----
            v0 = work.tile([LATENT, G], f32, tag="v0")
            nc.scalar.activation(out=v0, in_=dm, func=AF.Exp, bias=b2n_sb,
                                 scale=-1.0)
            nc.vector.tensor_scalar_add(v0, v0, 1.0)
            nc.vector.reciprocal(v0, v0)
            if dbg == 7:
                nc.sync.dma_start(out=d_xdbg[:], in_=v0)

            vc_ps = ps1.tile([128, 4, G], f32, tag="u_ps")
            for c in range(4):
                nc.tensor.transpose(vc_ps[:, c, :], cl_sb[:, c * 128:(c + 1) * 128],
                                    i128_sb[0:G, 0:G])
            vc = work.tile([128, 4, G], f32, tag="vc")
            nc.scalar.activation(out=vc, in_=vc_ps, func=AF.Exp, scale=-1.0)
            nc.vector.tensor_scalar_add(vc, vc, 1.0)
            nc.vector.reciprocal(vc, vc)
            if dbg == 8:
                nc.sync.dma_start(out=d_xdbg[:], in_=vc)

            # ---- decoder MLP (graphs on the free dim) ----
            h1_ps = ps1.tile([128, G], f32, tag="ft_ps")
            nc.tensor.matmul(h1_ps, w1_sb[:, 0, :], v0, start=True, stop=False)
            for c in range(4):
                nc.tensor.matmul(h1_ps, w1_sb[:, c + 1, :], vc[:, c, :],
                                 start=False, stop=(c == 3))
            h1 = work.tile([128, G], f32, tag="h1")
            nc.vector.tensor_scalar(h1, h1_ps, b1_sb, 0.0, OP.add, OP.max)
            if dbg == 9:
                nc.sync.dma_start(out=d_xdbg[:], in_=h1)

            h2_ps = ps1.tile([128, 2, G], f32, tag="u_ps")
            for b in range(2):
                nc.tensor.matmul(h2_ps[:, b, :], w2_sb[:, b, :], h1,
                                 start=True, stop=True)
            h2 = work.tile([128, 2, G], f32, tag="h2")
            for b in range(2):
                nc.vector.tensor_scalar(h2[:, b, :], h2_ps[:, b, :],
                                        b2d_sb[:, b:b + 1], 0.0, OP.add, OP.max)

            if dbg == 10:
                nc.sync.dma_start(out=d_xdbg[:], in_=h2)
            h3_ps = ps1.tile([128, 4, G], f32, tag="ft_ps")
            for b in range(4):
                for kc in range(2):
                    nc.tensor.matmul(h3_ps[:, b, :], w3_sb[:, kc, b, :],
                                     h2[:, kc, :], start=(kc == 0),
                                     stop=(kc == 1))
            h3 = work.tile([128, 4, G], f32, tag="h3")
            for b in range(4):
                nc.vector.tensor_scalar(h3[:, b, :], h3_ps[:, b, :],
                                        b3_sb[:, b:b + 1], 0.0, OP.add, OP.max)

            if dbg == 11:
                nc.sync.dma_start(out=d_xdbg[:], in_=h3)
            s_ps = ps1.tile([1, G], f32, tag="u_ps")
            for c in range(4):
                nc.tensor.matmul(s_ps, w4_sb[:, c:c + 1], h3[:, c, :],
                                 start=(c == 0), stop=(c == 3))
            s_sb = work.tile([1, G], f32, tag="s_sb")
            if dbg == 12:
                nc.vector.tensor_copy(s_sb, s_ps)
                nc.sync.dma_start(out=d_xdbg[:], in_=s_sb)
            else:
                nc.vector.tensor_scalar_add(s_sb, s_ps, b4_sb)
                if dbg == 13:
                    nc.sync.dma_start(out=d_xdbg[:], in_=s_sb)
                nc.sync.dma_start(out=d_score.rearrange("g x -> x g"),
                                  in_=s_sb[0:1, :])

    return nc


def _fix_preamble_regs(nc):
    """Bacc defers register allocation; its alloc_regs pass skips the
    framework preamble registers (*_zero, *_bcreg*, *_tpb_base*, monotonic),
    leaving reg_id=-1 which walrus rejects. Assign collision-free ids."""
    per_engine_used = {}
    pending = []
    for alloc in nc.m.functions[0].allocations:
        eng = getattr(alloc, "engine", None)
        rid = getattr(alloc, "reg_id", None)
        if eng is None or rid is None:
            continue
        if rid >= 0:
            per_engine_used.setdefault(eng, set()).add(rid)
        else:
            pending.append(alloc)
    canonical = {"zero": 8, "monotonic_0_cnt": 9, "bcreg0_lo": 10,
                 "bcreg0_hi": 11, "bcreg1_lo": 12, "bcreg1_hi": 13,
                 "tpb_base_lo": 14, "tpb_base_hi": 15}
    for alloc in pending:
        eng = alloc.engine
        used = per_engine_used.setdefault(eng, set())
        suffix = alloc.name.split("_", 1)[1] if "_" in alloc.name else alloc.name
        rid = canonical.get(suffix, 16)
        while rid in used:
            rid += 1
        alloc.reg_id = rid
        used.add(rid)


def _stage(inputs):
    """Host-side layout staging. Returns per-core in_maps."""
    xs = np.asarray(inputs["xs"], dtype=np.float32)
    A = np.asarray(inputs["A"])
    cell = np.asarray(inputs["cell_emb"], dtype=np.float32)
    Wg = np.asarray(inputs["Wg"], dtype=np.float32)
    bg = np.asarray(inputs["bg"], dtype=np.float32)
    attn = np.asarray(inputs["attn"], dtype=np.float32)
    Wt = np.asarray(inputs["Wt"], dtype=np.float32)
    bt = np.asarray(inputs["bt"], dtype=np.float32)
    Wf = np.asarray(inputs["Wf"], dtype=np.float32)
    bf = np.asarray(inputs["bf"], dtype=np.float32)
    Wf2 = np.asarray(inputs["Wf2"], dtype=np.float32)
    bf2 = np.asarray(inputs["bf2"], dtype=np.float32)
    W1 = np.asarray(inputs["W1"], dtype=np.float32)
    b1 = np.asarray(inputs["b1"], dtype=np.float32)
    W2 = np.asarray(inputs["W2"], dtype=np.float32)
    b2 = np.asarray(inputs["b2"], dtype=np.float32)
    W3 = np.asarray(inputs["W3"], dtype=np.float32)
    b3 = np.asarray(inputs["b3"], dtype=np.float32)
    W4 = np.asarray(inputs["W4"], dtype=np.float32)
    b4 = np.asarray(inputs["b4"], dtype=np.float32)

    wg_aug = np.concatenate([Wg, bg[:, None, :]], axis=1).copy()        # [3,35,34]
    a12 = attn.reshape(3, 2, ATOM).copy()                               # [3,2,34]
    wt_aug = np.concatenate([Wt, bt[None, :]], axis=0).copy()           # [35,256]
    wf_c = Wf.reshape(2, 128, ATOM).copy()                              # [2,128,34]
    b2n = -(bf @ Wf2 + bf2).reshape(LATENT, 1).copy()                   # [128,1]
    w1_c = W1.reshape(5, 128, 128).copy()
    w2_c = np.ascontiguousarray(W2.reshape(128, 2, 128).transpose(1, 0, 2))
    b2d_c = b2.reshape(2, 128, 1).copy()
    w3_c = np.ascontiguousarray(
        W3.reshape(2, 128, 4, 128).transpose(0, 2, 1, 3))               # [kc,b,128,128]
    b3_c = b3.reshape(4, 128, 1).copy()
    w4_c = W4.reshape(4, 128, 1).copy()
    b4_c = b4.reshape(1, 1).copy()
    i128 = np.eye(128, dtype=np.float32)

    shared = dict(wg=wg_aug, a12=a12, wt=wt_aug, wf=wf_c, wf2=Wf2.copy(),
                  b2n=b2n, w1=w1_c, b1=b1.reshape(128, 1).copy(), w2=w2_c,
                  b2d=b2d_c, w3=w3_c, b3=b3_c, w4=w4_c, b4=b4_c, i128=i128)

    in_maps = []
    for core in range(NCORES):
        sl = slice(core * G, (core + 1) * G)
        x0t = np.concatenate(
            [xs[sl].transpose(0, 2, 1),
             np.ones((G, 1, N), np.float32)], axis=1).copy()     # [8,35,128]
        m01t = np.ascontiguousarray(
            (A[sl] > 0).transpose(0, 2, 1).astype(np.float32))   # [8,128,128]
        m = dict(shared)
        m.update(x0t=x0t, m01t=m01t, cell=np.ascontiguousarray(cell[sl]))
        in_maps.append(m)
    return in_maps


def get_nc(dbg=0):
    key = f"nc{dbg}"
    if key not in _CACHE:
        nc = _build_bass(dbg)
        nc.finalize()
        _fix_preamble_regs(nc)
        _CACHE[key] = nc
    return _CACHE[key]


def kernel(**inputs) -> np.ndarray:
    from concourse.bass_utils import run_bass_kernel_spmd

    nc = get_nc()
    in_maps = _stage(inputs)
    res = run_bass_kernel_spmd(nc, in_maps, core_ids=list(range(NCORES)))
    out = np.concatenate([res.results[i]["score"] for i in range(NCORES)], axis=0)
    return out.astype(np.float32)



# revision 11
# speedup vs baseline: 1.9135x; 1.9135x over previous
"""Trainium2 Bass kernel for nn_DiseaseModel_mlp (GNN message passing + MLP decoder).

Data parallel over batch: 64 graphs -> 8 NeuronCores x 8 graphs; weights
replicated. Per core the 8 graphs are split into 2 groups of 4 that are
pipelined across engines (PE / ACT / DVE / Pool).

Key structure (fp16 on device, fp32 PSUM accumulation):
- state kept feature-major [35, G, 128] (row 34 = ones -> bias folding)
- per round: ht = relu(Wg^T @ state) (feature-major), h = relu(state^T Wg)
  (node-major), f = a12aug^T @ [ht;1] gives rows [f_dst, ones, f_src] in
  one matmul; e[q,p] = f_dst[q]+f_src[p] via one rank-2 matmul per graph;
  lrelu via ACT Prelu(alpha=.01) in-place on PSUM; mask folded as -25 on
  masked entries via an fp8 identity x moff matmul accumulated onto the
  same PSUM; exp on ACT; U = P^T @ [h|1] gives messages + softmax
  row-sums in one matmul. No DVE mask multiply, no max-subtraction.
- decoder cell_emb branch (4/5 of W1) precomputed during round 0.

Note: every matmul operand is kept at SBUF base partition 0 (operands at
other bases can hard-crash the device).
"""

import numpy as np

ATOM = 34
HID = 256
LATENT = 128
CELLS = 512
B, N = 64, 128
NCORES = 8
G = B // NCORES          # graphs per core = 8
NG = 4                   # graphs per pipeline group
MOFF = -25.0             # additive mask offset (post-lrelu), exp(-25+6)~0
N_WARM_MM = 12           # dummy matmuls to ramp the PE p-state during DMA

_CACHE = {}


def _build_bass(dbg=0):
    import concourse.bass as bass
    import concourse.bacc as bacc
    import concourse.mybir as mybir
    import concourse.tile as tile

    f32 = mybir.dt.float32
    f16 = mybir.dt.float16
    f8 = mybir.dt.float8e4
    AF = mybir.ActivationFunctionType
    OP = mybir.AluOpType
    AX = mybir.AxisListType

    nc = bacc.Bacc(None)

    FA = 3 * ATOM + 9 + 128                     # wg(102) a12aug(9) i128(128)
    FB = 256 + 68 + 128 + 640 + 256 + 1024 + 4

    d_x0t = nc.declare_dram_parameter("x0t", [ATOM + 1, G, N], f16, isOutput=False)
    d_mo8 = nc.declare_dram_parameter("mo8", [128, G * N + 128], f8, isOutput=False)
    d_cellT = nc.declare_dram_parameter("cellT", [128, 4, G], f16, isOutput=False)
    d_wA = nc.declare_dram_parameter("wA", [128, FA], f16, isOutput=False)
    d_wB = nc.declare_dram_parameter("wB", [128, FB], f16, isOutput=False)
    d_b32 = nc.declare_dram_parameter("b32", [128, 9], f32, isOutput=False)
    d_score = nc.declare_dram_parameter("score", [G, 1], f32, isOutput=True)

    _dbgshape = {1: [ATOM, G, N], 2: [ATOM, G, N], 3: [ATOM, G, N],
                 4: [ATOM, G, N], 5: [128, G, N], 6: [3, G, N],
                 7: [128, G, ATOM + 1], 8: [128, G, ATOM], 9: [128, 2, G, N],
                 10: [ATOM, G, N], 11: [128, G], 12: [128, G],
                 13: [128, G], 14: [128, 2, G], 15: [1, G]}.get(dbg, [1, 1])
    d_xdbg = (nc.declare_dram_parameter("xdbg", _dbgshape, f32, isOutput=True)
              if dbg else None)

    groups = [(0, slice(0, NG)), (1, slice(NG, G))]

    lowp = nc.allow_low_precision(reason="fp16 pipeline; tolerance 2e-2")
    lowp.__enter__()
    with tile.TileContext(nc) as tc:
        with (
            tc.tile_pool(name="singles", bufs=1) as singles,
            tc.tile_pool(name="work", bufs=2) as work,
            tc.tile_pool(name="pbig", bufs=1, space="PSUM") as pbig,
            tc.tile_pool(name="patt", bufs=1, space="PSUM") as patt,
            tc.tile_pool(name="psmall", bufs=1, space="PSUM") as psmall,
            tc.tile_pool(name="pdT", bufs=1, space="PSUM") as pdT,
            tc.tile_pool(name="pdeco", bufs=1, space="PSUM") as pdeco,
        ):
            # ---------------- input DMAs (issue ASAP, big ones first) ------
            state0 = singles.tile([ATOM + 1, G, N], f16, tag="state0")
            nc.sync.dma_start(out=state0[:, 0:NG, :], in_=d_x0t[:, 0:NG, :])
            wA_sb = singles.tile([128, FA], f16, tag="wA")
            nc.scalar.dma_start(out=wA_sb, in_=d_wA[:])
            nc.sync.dma_start(out=state0[:, NG:G, :], in_=d_x0t[:, NG:G, :])
            mo8_sb = singles.tile([128, G * N + 128], f8, tag="mo8")
            nc.scalar.dma_start(out=mo8_sb[:, 0:NG * N], in_=d_mo8[:, 0:NG * N])
            nc.sync.dma_start(out=mo8_sb[:, NG * N:], in_=d_mo8[:, NG * N:])
            b32_sb = singles.tile([128, 9], f32, tag="b32")
            nc.scalar.dma_start(out=b32_sb, in_=d_b32[:])
            cellT_sb = singles.tile([128, 4, G], f16, tag="cellT")
            nc.sync.dma_start(out=cellT_sb, in_=d_cellT[:])
            wB_sb = singles.tile([128, FB], f16, tag="wB")
            nc.scalar.dma_start(out=wB_sb, in_=d_wB[:])

            # weight views inside the blobs
            wg = wA_sb[0:ATOM + 1, 0:3 * ATOM].rearrange(
                "p (r d) -> p r d", r=3)
            a12 = wA_sb[0:ATOM + 1, 3 * ATOM:3 * ATOM + 9].rearrange(
                "p (r c) -> p r c", r=3)
            i128 = wA_sb[:, 3 * ATOM + 9:3 * ATOM + 9 + 128]
            o = 0
            wtaug = wB_sb[0:ATOM + 1, o:o + 256].rearrange(
                "p (h m) -> p h m", h=2); o += 256
            wf = wB_sb[:, o:o + 68].rearrange("p (k d) -> p k d", k=2); o += 68
            wf2 = wB_sb[0:ATOM, o:o + 128]; o += 128
            w1 = wB_sb[:, o:o + 640].rearrange("p (k m) -> p k m", k=5); o += 640
            w2 = wB_sb[:, o:o + 256].rearrange("p (b m) -> p b m", b=2); o += 256
            w3 = wB_sb[:, o:o + 1024].rearrange(
                "p (k b m) -> p k b m", k=2, b=4); o += 1024
            w4 = wB_sb[:, o:o + 4]; o += 4
            b2n = b32_sb[:, 0:1]
            b1p = b32_sb[:, 1:2]
            b2d = b32_sb[:, 2:4]
            b3p = b32_sb[:, 4:8]
            b4p = b32_sb[0:1, 8:9]
            eye8 = mo8_sb[:, G * N:]

            # ---------------- static SBUF tiles ---------------------------
            # ACT table warm-up (Exp/Relu/Prelu/Copy share one table set)
            warm = singles.tile([1, 1], f32, tag="warm")
            nc.vector.memset(warm, 0.0)
            nc.scalar.activation(out=warm, in_=warm, func=AF.Exp)

            htaug = singles.tile([ATOM + 1, G, N], f16, tag="htaug")
            nc.vector.memset(htaug[ATOM:ATOM + 1], 1.0)
            haug = singles.tile([128, G, ATOM + 1], f16, tag="haug")
            nc.vector.memset(haug[:, :, ATOM], 1.0)
            f3 = singles.tile([3, G, N], f16, tag="f3")
            f3b = singles.tile([2, G, N], f16, tag="f3b")
            nc.vector.memset(f3b[0:1], 1.0)
            P_sb = singles.tile([128, G, N], f16, tag="P_sb")
            inv = singles.tile([128, G], f32, tag="inv")
            dlt = singles.tile([128, G, ATOM], f16, tag="dlt")
            h1c_sb = singles.tile([128, G], f32, tag="h1c_sb")
            states = [state0]
            for r in range(3):
                st = singles.tile([ATOM + 1, G, N], f16, tag=f"state{r + 1}")
                nc.gpsimd.memset(st[ATOM:ATOM + 1], 1.0)
                states.append(st)

            # PE warm-up: dummy matmuls (no DMA deps) ramp the p-state
            zz = singles.tile([1, 128], f16, tag="zz")
            nc.vector.memset(zz, 0.0)
            deco_ps = pdeco.tile([128, 128], f32, tag="deco")
            wm_ps = deco_ps[0:1, 0:128]
            for i in range(N_WARM_MM):
                nc.tensor.matmul(wm_ps, zz[0:1, 0:1], zz, start=True, stop=True)

            # ---------------- GNN rounds ----------------------------------
            for r in range(3):
                S = states[r]
                Snx = states[r + 1]
                for gi, sl in groups:
                    ga = gi * NG
                    # ht = Wg^T @ state  (feature-major h, pre-relu, PSUM)
                    ht_ps = pbig.tile([ATOM, NG, N], f32, tag="ht")
                    nc.tensor.matmul(ht_ps, wg[:, r, :], S[:, sl, :],
                                     start=True, stop=True)
                    # h node-major per graph
                    hu_ps = psmall.tile([128, NG, 70], f32, tag="hu")
                    h_ps = hu_ps[:, :, 0:ATOM]
                    for g in range(NG):
                        nc.tensor.matmul(h_ps[:, g, :], S[:, ga + g, :],
                                         wg[:, r, :], start=True, stop=True)
                    # evacuate with relu: ht -> htaug (ACT), h -> haug (DVE)
                    nc.scalar.activation(out=htaug[0:ATOM, sl, :], in_=ht_ps,
                                         func=AF.Relu)
                    nc.vector.tensor_scalar_max(haug[:, sl, 0:ATOM], h_ps, 0.0)
                    # f = a12aug^T @ [ht;1] -> rows [f_dst, ones, f_src]
                    f_ps = pbig.tile([3, NG, N], f32, tag="f")
                    nc.tensor.matmul(f_ps, a12[:, r, :], htaug[:, sl, :],
                                     start=True, stop=True)
                    nc.vector.tensor_copy(f3[:, sl, :], f_ps)
                    # second operand tile [ones; f_src] (base-0 safety copy)
                    nc.vector.tensor_copy(f3b[1:2, sl, :], f_ps[2:3])
                    # e[q,p] = f_dst[q] + f_src[p]  (rank-2 matmul per graph)
                    e_ps = patt.tile([128, NG, N], f32, tag="e", bufs=2)
                    for g in range(NG):
                        nc.tensor.matmul(e_ps[:, g, :], f3[0:2, ga + g, :],
                                         f3b[:, ga + g, :], start=True, stop=True)
                    # t = lrelu(e)  (ACT Prelu, in-place on PSUM)
                    nc.scalar.activation(out=e_ps, in_=e_ps, func=AF.Prelu,
                                         alpha=0.01)
                    # t += moff (0 / -25) via fp8 identity matmul accumulate
                    for g in range(NG):
                        nc.tensor.matmul(e_ps[:, g, :], eye8,
                                         mo8_sb[:, (ga + g) * N:(ga + g + 1) * N],
                                         start=False, stop=True,
                                         skip_group_check=True)
                    # P = exp(t) -> fp16 SBUF
                    nc.scalar.activation(out=P_sb[:, sl, :], in_=e_ps, func=AF.Exp)
                    # U = P^T @ [h|1] : messages + row-sums
                    u_ps = hu_ps[:, :, ATOM:2 * ATOM + 1]
                    for g in range(NG):
                        nc.tensor.matmul(u_ps[:, g, :], P_sb[:, ga + g, :],
                                         haug[:, ga + g, :], start=True, stop=True)
                    nc.vector.reciprocal(inv[:, sl], u_ps[:, :, ATOM])
                    i_b = inv[:, sl].unsqueeze(2).to_broadcast([128, NG, ATOM])
                    nc.vector.tensor_tensor(dlt[:, sl, :], u_ps[:, :, 0:ATOM],
                                            i_b, OP.mult)
                    # delta^T per graph, then state update on Pool
                    dT_ps = pdT.tile([ATOM, NG, N], f16, tag="dT")
                    for g in range(NG):
                        nc.tensor.transpose(dT_ps[:, g, :], dlt[:, ga + g, :],
                                            i128)
                    nc.gpsimd.tensor_tensor(Snx[0:ATOM, sl, :], S[0:ATOM, sl, :],
                                            dT_ps, OP.add)
                if dbg == r + 1:
                    xd = work.tile([ATOM, G, N], f32, tag="xd")
                    nc.vector.tensor_copy(xd, states[r + 1][0:ATOM])
                    nc.sync.dma_start(out=d_xdbg[:], in_=xd)
                if dbg in (4, 5, 6, 7, 8) and r == 0:
                    src = {4: htaug[0:ATOM], 5: P_sb, 6: f3,
                           7: haug, 8: dlt}[dbg]
                    xd = work.tile(_dbgshape, f32, tag="xd")
                    nc.vector.tensor_copy(xd, src)
                    nc.sync.dma_start(out=d_xdbg[:], in_=xd)

                # cell-branch of the decoder during round 0 (idle slots)
                if r == 0:
                    vc = singles.tile([128, 4, G], f16, tag="vc")
                    nc.scalar.activation(out=vc, in_=cellT_sb, func=AF.Exp,
                                         scale=-1.0)
                    nc.vector.tensor_scalar_add(vc, vc, 1.0)
                    nc.vector.reciprocal(vc, vc)
                    h1c_ps = deco_ps[:, 0:G]
                    for c in range(4):
                        nc.tensor.matmul(h1c_ps, w1[:, c + 1, :], vc[:, c, :],
                                         start=(c == 0), stop=(c == 3))
                    nc.vector.tensor_copy(h1c_sb, h1c_ps)

            # ---------------- VEC head + decoder --------------------------
            S3 = states[3]
            gts = singles.tile([128, 2, G, N], f16, tag="gts")
            d1_sb = singles.tile([ATOM, G, N], f16, tag="d1_sb")
            dm = singles.tile([128, G], f32, tag="dm")
            v0 = singles.tile([128, G], f16, tag="v0")
            for gi, sl in groups:
                # g = relu(Wt^T @ state3), two 128-halves
                for hh in range(2):
                    gt_ps = patt.tile([128, NG, N], f32, tag="e", bufs=2)
                    nc.tensor.matmul(gt_ps, wtaug[:, hh, :], S3[:, sl, :],
                                     start=True, stop=True)
                    if hh == 0:
                        nc.scalar.activation(out=gts[:, hh, sl, :], in_=gt_ps,
                                             func=AF.Relu)
                    else:
                        nc.vector.tensor_scalar_max(gts[:, hh, sl, :], gt_ps, 0.0)
                # d1 = Wf^T @ g + x0   (residual via identity matmul)
                d1_ps = pdT.tile([ATOM, NG, N], f32, tag="d1")
                for k in range(2):
                    nc.tensor.matmul(d1_ps, wf[:, k, :], gts[:, k, sl, :],
                                     start=(k == 0), stop=False)
                nc.tensor.matmul(d1_ps, i128[0:ATOM, 0:ATOM],
                                 state0[0:ATOM, sl, :], start=False, stop=True)
                nc.scalar.activation(out=d1_sb[:, sl, :], in_=d1_ps, func=AF.Copy)
                # d2 = Wf2^T @ d1 ; dm = max over nodes
                d2_ps = patt.tile([128, NG, N], f32, tag="e", bufs=2)
                nc.tensor.matmul(d2_ps, wf2, d1_sb[:, sl, :],
                                 start=True, stop=True)
                nc.vector.tensor_reduce(dm[:, sl], d2_ps, AX.X, OP.max)
                # v0 = sigmoid(dm + bias) = 1/(1+exp(-dm+b2n))
                nc.scalar.activation(out=v0[:, sl], in_=dm[:, sl], func=AF.Exp,
                                     bias=b2n, scale=-1.0)
                nc.vector.tensor_scalar_add(v0[:, sl], v0[:, sl], 1.0)
                nc.vector.reciprocal(v0[:, sl], v0[:, sl])
            if dbg == 9:
                xd = work.tile([128, 2, G, N], f32, tag="xd")
                nc.vector.tensor_copy(xd, gts)
                nc.sync.dma_start(out=d_xdbg[:], in_=xd)
            if dbg == 10:
                xd = work.tile([ATOM, G, N], f32, tag="xd")
                nc.vector.tensor_copy(xd, d1_sb)
                nc.sync.dma_start(out=d_xdbg[:], in_=xd)
            if dbg == 11:
                nc.sync.dma_start(out=d_xdbg[:], in_=dm)
            if dbg == 12:
                xd = work.tile([128, G], f32, tag="xd")
                nc.vector.tensor_copy(xd, v0)
                nc.sync.dma_start(out=d_xdbg[:], in_=xd)

            # h1 = relu(W1g^T v0 + h1c + b1)
            h1_ps = deco_ps[:, 8:8 + G]
            nc.tensor.matmul(h1_ps, w1[:, 0, :], v0, start=True, stop=True)
            h1 = work.tile([128, G], f16, tag="h1")
            nc.vector.scalar_tensor_tensor(h1, h1_ps, b1p, h1c_sb,
                                           OP.add, OP.add)
            nc.vector.tensor_scalar_max(h1, h1, 0.0)
            if dbg == 13:
                xd = work.tile([128, G], f32, tag="xd")
                nc.vector.tensor_copy(xd, h1)
                nc.sync.dma_start(out=d_xdbg[:], in_=xd)

            h2_ps = deco_ps[:, 16:32].rearrange("p (b g) -> p b g", b=2)
            for b in range(2):
                nc.tensor.matmul(h2_ps[:, b, :], w2[:, b, :], h1,
                                 start=True, stop=True)
            h2 = work.tile([128, 2, G], f16, tag="h2")
            for b in range(2):
                nc.vector.tensor_scalar(h2[:, b, :], h2_ps[:, b, :],
                                        b2d[:, b:b + 1], 0.0, OP.add, OP.max)
            if dbg == 14:
                xd = work.tile([128, 2, G], f32, tag="xd")
                nc.vector.tensor_copy(xd, h2)
                nc.sync.dma_start(out=d_xdbg[:], in_=xd)

            h3_ps = deco_ps[:, 32:64].rearrange("p (b g) -> p b g", b=4)
            for b in range(4):
                for kc in range(2):
                    nc.tensor.matmul(h3_ps[:, b, :], w3[:, kc, b, :],
                                     h2[:, kc, :], start=(kc == 0),
                                     stop=(kc == 1))
            h3 = work.tile([128, 4, G], f16, tag="h3")
            for b in range(4):
                nc.vector.tensor_scalar(h3[:, b, :], h3_ps[:, b, :],
                                        b3p[:, b:b + 1], 0.0, OP.add, OP.max)

            s_ps = deco_ps[0:1, 64:64 + G]
            for c in range(4):
                nc.tensor.matmul(s_ps, w4[:, c:c + 1], h3[:, c, :],
                                 start=(c == 0), stop=(c == 3))
            s_sb = work.tile([1, G], f32, tag="s_sb")
            nc.vector.tensor_scalar_add(s_sb, s_ps, b4p)
            if dbg == 15:
                nc.sync.dma_start(out=d_xdbg[:], in_=s_sb)
            nc.sync.dma_start(out=d_score.rearrange("g x -> x g"), in_=s_sb)

    lowp.__exit__(None, None, None)
    return nc


def _fix_preamble_regs(nc):
    """Bacc defers register allocation; its alloc_regs pass skips the
    framework preamble registers, leaving reg_id=-1 which walrus rejects.
    Assign collision-free ids."""
    per_engine_used = {}
    pending = []
    for alloc in nc.m.functions[0].allocations:
        eng = getattr(alloc, "engine", None)
        rid = getattr(alloc, "reg_id", None)
        if eng is None or rid is None:
            continue
        if rid >= 0:
            per_engine_used.setdefault(eng, set()).add(rid)
        else:
            pending.append(alloc)
    canonical = {"zero": 8, "monotonic_0_cnt": 9, "bcreg0_lo": 10,
                 "bcreg0_hi": 11, "bcreg1_lo": 12, "bcreg1_hi": 13,
                 "tpb_base_lo": 14, "tpb_base_hi": 15}
    for alloc in pending:
        eng = alloc.engine
        used = per_engine_used.setdefault(eng, set())
        suffix = alloc.name.split("_", 1)[1] if "_" in alloc.name else alloc.name
        rid = canonical.get(suffix, 16)
        while rid in used:
            rid += 1
        alloc.reg_id = rid
        used.add(rid)


def _stage(inputs):
    """Host-side layout staging (fp16/fp8 packing). Returns per-core in_maps."""
    import ml_dtypes

    f16 = np.float16
    xs = np.asarray(inputs["xs"], dtype=np.float32)
    A = np.asarray(inputs["A"])
    cell = np.asarray(inputs["cell_emb"], dtype=np.float32)
    Wg = np.asarray(inputs["Wg"], dtype=np.float32)
    bg = np.asarray(inputs["bg"], dtype=np.float32)
    attn = np.asarray(inputs["attn"], dtype=np.float32)
    Wt = np.asarray(inputs["Wt"], dtype=np.float32)
    bt = np.asarray(inputs["bt"], dtype=np.float32)
    Wf = np.asarray(inputs["Wf"], dtype=np.float32)
    bf = np.asarray(inputs["bf"], dtype=np.float32)
    Wf2 = np.asarray(inputs["Wf2"], dtype=np.float32)
    bf2 = np.asarray(inputs["bf2"], dtype=np.float32)
    W1 = np.asarray(inputs["W1"], dtype=np.float32)
    b1 = np.asarray(inputs["b1"], dtype=np.float32)
    W2 = np.asarray(inputs["W2"], dtype=np.float32)
    b2 = np.asarray(inputs["b2"], dtype=np.float32)
    W3 = np.asarray(inputs["W3"], dtype=np.float32)
    b3 = np.asarray(inputs["b3"], dtype=np.float32)
    W4 = np.asarray(inputs["W4"], dtype=np.float32)
    b4 = np.asarray(inputs["b4"], dtype=np.float32)

    FA = 3 * ATOM + 9 + 128
    wA = np.zeros((128, FA), np.float32)
    wA[:ATOM, 0:3 * ATOM] = Wg.transpose(1, 0, 2).reshape(ATOM, 3 * ATOM)
    wA[ATOM, 0:3 * ATOM] = bg.reshape(3 * ATOM)
    a12aug = np.zeros((ATOM + 1, 3, 3), np.float32)
    for r in range(3):
        a12aug[:ATOM, r, 0] = attn[r, ATOM:]
        a12aug[ATOM, r, 1] = 1.0
        a12aug[:ATOM, r, 2] = attn[r, :ATOM]
    wA[:ATOM + 1, 3 * ATOM:3 * ATOM + 9] = a12aug.reshape(ATOM + 1, 9)
    wA[:, 3 * ATOM + 9:] = np.eye(128, dtype=np.float32)

    FB = 256 + 68 + 128 + 640 + 256 + 1024 + 4
    wB = np.zeros((128, FB), np.float32)
    o = 0
    wB[:ATOM, o:o + 256] = Wt
    wB[ATOM, o:o + 256] = bt
    o += 256
    wB[:, o:o + 68] = Wf.reshape(2, 128, ATOM).transpose(1, 0, 2).reshape(128, 68)
    o += 68
    wB[:ATOM, o:o + 128] = Wf2
    o += 128
    wB[:, o:o + 640] = W1.reshape(5, 128, 128).transpose(1, 0, 2).reshape(128, 640)
    o += 640
    wB[:, o:o + 256] = W2.reshape(128, 2, 128).reshape(128, 256)
    o += 256
    wB[:, o:o + 1024] = W3.reshape(2, 128, 4, 128).transpose(1, 0, 2, 3).reshape(128, 1024)
    o += 1024
    wB[:, o:o + 4] = W4.reshape(4, 128).T
    o += 4

    b32 = np.zeros((128, 9), np.float32)
    b32[:, 0] = -(bf @ Wf2 + bf2)
    b32[:, 1] = b1
    b32[:, 2:4] = b2.reshape(2, 128).T
    b32[:, 4:8] = b3.reshape(4, 128).T
    b32[0, 8] = b4[0]

    shared = dict(wA=wA.astype(f16), wB=wB.astype(f16), b32=b32)

    in_maps = []
    for core in range(NCORES):
        sl = slice(core * G, (core + 1) * G)
        x0t = np.concatenate(
            [xs[sl].transpose(0, 2, 1),
             np.ones((G, 1, N), np.float32)], axis=1)      # [G, 35, N]
        x0t = np.ascontiguousarray(x0t.transpose(1, 0, 2)).astype(f16)
        # moff[q, g, p] = 0 if A[g, p, q] > 0 else MOFF ; plus eye cols
        m_qgp = (np.asarray(A[sl]) > 0).transpose(2, 0, 1)  # [q, g, p]
        mo = np.where(m_qgp, 0.0, MOFF).astype(np.float32).reshape(128, G * N)
        mo8 = np.concatenate([mo, np.eye(128, dtype=np.float32)], axis=1)
        cellT = np.ascontiguousarray(
            cell[sl].reshape(G, 4, 128).transpose(2, 1, 0)).astype(f16)
        m = dict(shared)
        m.update(x0t=x0t,
                 mo8=mo8.astype(ml_dtypes.float8_e4m3),
                 cellT=cellT)
        in_maps.append(m)
    return in_maps


def get_nc(dbg=0):
    key = f"nc{dbg}"
    if key not in _CACHE:
        nc = _build_bass(dbg)
        nc.finalize()
        _fix_preamble_regs(nc)
        _CACHE[key] = nc
    return _CACHE[key]


def kernel(**inputs) -> np.ndarray:
    from concourse.bass_utils import run_bass_kernel_spmd

    nc = get_nc()
    in_maps = _stage(inputs)
    res = run_bass_kernel_spmd(nc, in_maps, core_ids=list(range(NCORES)))
    out = np.concatenate([res.results[i]["score"] for i in range(NCORES)], axis=0)
    return out.astype(np.float32)


# revision 13
# speedup vs baseline: 2.0548x; 1.0738x over previous
"""Trainium2 Bass kernel for nn_DiseaseModel_mlp (GNN message passing + MLP decoder).

Data parallel over batch: 64 graphs -> 8 NeuronCores x 8 graphs; weights
replicated. Per core the 8 graphs are split into 2 groups of 4 that are
pipelined across engines (PE / ACT / DVE / Pool).

Key structure (fp16 on device, fp32 PSUM accumulation):
- state kept feature-major [35, G, 128] (row 34 = ones -> bias folding)
- per round: ht = relu(Wg^T @ state) (feature-major), h = relu(state^T Wg)
  (node-major), f = a12aug^T @ [ht;1] gives rows [f_dst, ones, f_src] in
  one matmul; e[q,p] = f_dst[q]+f_src[p] via one rank-2 matmul per graph;
  lrelu via ACT Prelu(alpha=.01) in-place on PSUM; mask folded as -25 on
  masked entries via an fp8 identity x moff matmul accumulated onto the
  same PSUM; exp on ACT; U = P^T @ [h|1] gives messages + softmax
  row-sums in one matmul. No DVE mask multiply, no max-subtraction.
- decoder cell_emb branch (4/5 of W1) precomputed during round 0.

Note: every matmul operand is kept at SBUF base partition 0 (operands at
other bases can hard-crash the device).
"""

import numpy as np

ATOM = 34
HID = 256
LATENT = 128
CELLS = 512
B, N = 64, 128
NCORES = 8
G = B // NCORES          # graphs per core = 8
NG = 4                   # graphs per pipeline group
MOFF = -25.0             # additive mask offset (post-lrelu), exp(-25+6)~0
N_WARM_MM = 12           # dummy matmuls to ramp the PE p-state during DMA

_CACHE = {}


def _build_bass(dbg=0):
    import concourse.bass as bass
    import concourse.bacc as bacc
    import concourse.mybir as mybir
    import concourse.tile as tile

    f32 = mybir.dt.float32
    f16 = mybir.dt.float16
    f8 = mybir.dt.float8e4
    AF = mybir.ActivationFunctionType
    OP = mybir.AluOpType
    AX = mybir.AxisListType

    nc = bacc.Bacc(None)

    FA = 3 * 35 + 12 + 128                      # wg4(105) fLR(12) i128(128)
    FB = 256 + 68 + 128 + 640 + 256 + 1024 + 4

    d_x0t = nc.declare_dram_parameter("x0t", [ATOM + 1, G, N], f16, isOutput=False)
    d_mo8 = nc.declare_dram_parameter("mo8", [128, G * N + 128], f8, isOutput=False)
    d_cellT = nc.declare_dram_parameter("cellT", [128, 4, G], f16, isOutput=False)
    d_wA = nc.declare_dram_parameter("wA", [128, FA], f16, isOutput=False)
    d_wB = nc.declare_dram_parameter("wB", [128, FB], f16, isOutput=False)
    d_b32 = nc.declare_dram_parameter("b32", [128, 9], f32, isOutput=False)
    d_score = nc.declare_dram_parameter("score", [G, 1], f32, isOutput=True)

    _dbgshape = {1: [ATOM, G, N], 2: [ATOM, G, N], 3: [ATOM, G, N],
                 4: [ATOM, G, N], 5: [128, G, N], 6: [2, G, N],
                 7: [128, G, ATOM + 1], 8: [128, G, ATOM], 9: [128, 2, G, N],
                 10: [ATOM, G, N], 11: [128, G], 12: [128, G],
                 13: [128, G], 14: [128, 2, G], 15: [1, G]}.get(dbg, [1, 1])
    d_xdbg = (nc.declare_dram_parameter("xdbg", _dbgshape, f32, isOutput=True)
              if dbg else None)

    groups = [(0, slice(0, NG)), (1, slice(NG, G))]

    lowp = nc.allow_low_precision(reason="fp16 pipeline; tolerance 2e-2")
    lowp.__enter__()
    with tile.TileContext(nc) as tc:
        with (
            tc.tile_pool(name="singles", bufs=1) as singles,
            tc.tile_pool(name="work", bufs=2) as work,
            tc.tile_pool(name="pbig", bufs=1, space="PSUM") as pbig,
            tc.tile_pool(name="patt", bufs=1, space="PSUM") as patt,
            tc.tile_pool(name="pmisc", bufs=1, space="PSUM") as pmisc,
            tc.tile_pool(name="pdT", bufs=1, space="PSUM") as pdT,
        ):
            # ---------------- input DMAs (issue ASAP, big ones first) ------
            state0 = singles.tile([ATOM + 1, G, N], f16, tag="state0")
            nc.sync.dma_start(out=state0[:, 0:NG, :], in_=d_x0t[:, 0:NG, :])
            wA_sb = singles.tile([128, FA], f16, tag="wA")
            nc.scalar.dma_start(out=wA_sb, in_=d_wA[:])
            nc.sync.dma_start(out=state0[:, NG:G, :], in_=d_x0t[:, NG:G, :])
            mo8_sb = singles.tile([128, G * N + 128], f8, tag="mo8")
            nc.scalar.dma_start(out=mo8_sb[:, 0:NG * N], in_=d_mo8[:, 0:NG * N])
            nc.sync.dma_start(out=mo8_sb[:, NG * N:], in_=d_mo8[:, NG * N:])
            b32_sb = singles.tile([128, 9], f32, tag="b32")
            nc.scalar.dma_start(out=b32_sb, in_=d_b32[:])
            cellT_sb = singles.tile([128, 4, G], f16, tag="cellT")
            nc.sync.dma_start(out=cellT_sb, in_=d_cellT[:])
            wB_sb = singles.tile([128, FB], f16, tag="wB")
            nc.scalar.dma_start(out=wB_sb, in_=d_wB[:])

            # weight views inside the blobs
            wg = wA_sb[0:ATOM + 1, 0:105].rearrange(
                "p (r d) -> p r d", r=3)
            fLR = wA_sb[0:ATOM + 1, 105:117].rearrange(
                "p (r c) -> p r c", r=3)
            i128 = wA_sb[:, 117:117 + 128]
            o = 0
            wtaug = wB_sb[0:ATOM + 1, o:o + 256].rearrange(
                "p (h m) -> p h m", h=2); o += 256
            wf = wB_sb[:, o:o + 68].rearrange("p (k d) -> p k d", k=2); o += 68
            wf2 = wB_sb[0:ATOM, o:o + 128]; o += 128
            w1 = wB_sb[:, o:o + 640].rearrange("p (k m) -> p k m", k=5); o += 640
            w2 = wB_sb[:, o:o + 256].rearrange("p (b m) -> p b m", b=2); o += 256
            w3 = wB_sb[:, o:o + 1024].rearrange(
                "p (k b m) -> p k b m", k=2, b=4); o += 1024
            w4 = wB_sb[:, o:o + 4]; o += 4
            b2n = b32_sb[:, 0:1]
            b1p = b32_sb[:, 1:2]
            b2d = b32_sb[:, 2:4]
            b3p = b32_sb[:, 4:8]
            b4p = b32_sb[0:1, 8:9]
            eye8 = mo8_sb[:, G * N:]

            # ---------------- static SBUF tiles ---------------------------
            # ACT table warm-up (Exp/Relu/Prelu/Copy share one table set)
            warm = singles.tile([1, 1], f32, tag="warm")
            nc.vector.memset(warm, 0.0)
            nc.scalar.activation(out=warm, in_=warm, func=AF.Exp)

            htaug = singles.tile([ATOM + 1, G, N], f16, tag="htaug")
            haug = singles.tile([128, G, ATOM + 1], f16, tag="haug")
            nc.vector.memset(haug[:, :, ATOM], 1.0)
            f3 = singles.tile([2, G, N], f16, tag="f3")
            f3b = singles.tile([2, G, N], f16, tag="f3b")
            P_sb = singles.tile([128, G, N], f16, tag="P_sb")
            inv = singles.tile([128, G], f32, tag="inv")
            dlt = singles.tile([128, G, ATOM], f16, tag="dlt")
            h1c_sb = singles.tile([128, G], f32, tag="h1c_sb")
            states = [state0]
            for r in range(3):
                st = singles.tile([ATOM + 1, G, N], f16, tag=f"state{r + 1}")
                states.append(st)
            # shared delta^T PSUM tile; row 34 stays 0 so the state update
            # can add all 35 rows (ones row survives: 1 + 0)
            dT_ps = pdT.tile([ATOM + 1, G, N], f16, tag="dT")
            nc.vector.memset(dT_ps, 0.0)

            # PE warm-up: dummy matmuls (no DMA deps) ramp the p-state
            zz = singles.tile([1, 128], f16, tag="zz")
            nc.vector.memset(zz, 0.0)
            misc_ps = pmisc.tile([128, 408], f32, tag="misc")
            hu_all = misc_ps[:, 0:280].rearrange("p (g c) -> p g c", g=NG)
            deco_ps = misc_ps[:, 280:408]
            wm_ps = deco_ps[0:1, 0:128]
            for i in range(N_WARM_MM):
                nc.tensor.matmul(wm_ps, zz[0:1, 0:1], zz, start=True, stop=True)

            # ---------------- GNN rounds ----------------------------------
            for r in range(3):
                S = states[r]
                Snx = states[r + 1]
                for gi, sl in groups:
                    ga = gi * NG
                    # ht = Wg^T @ state (feature-major h; row 34 = ones
                    # via the extra e34 column of wg4)
                    ht_ps = pbig.tile([ATOM + 1, NG, N], f32, tag="ht")
                    nc.tensor.matmul(ht_ps, wg[:, r, :], S[:, sl, :],
                                     start=True, stop=True)
                    # h node-major per graph
                    hu_ps = hu_all
                    h_ps = hu_ps[:, :, 0:ATOM]
                    for g in range(NG):
                        nc.tensor.matmul(h_ps[:, g, :], S[:, ga + g, :],
                                         wg[:, r, 0:ATOM], start=True, stop=True)
                    # evacuate with relu: ht -> htaug (ACT), h -> haug (DVE)
                    nc.scalar.activation(out=htaug[:, sl, :], in_=ht_ps,
                                         func=AF.Relu)
                    nc.vector.tensor_scalar_max(haug[:, sl, 0:ATOM], h_ps, 0.0)
                    # f matmuls: fL -> [f_dst; ones], fR -> [ones; f_src]
                    f_ps = pbig.tile([2, NG, N], f32, tag="f")
                    nc.tensor.matmul(f_ps, fLR[:, r, 0:2], htaug[:, sl, :],
                                     start=True, stop=True)
                    nc.vector.tensor_copy(f3[:, sl, :], f_ps)
                    f2_ps = pbig.tile([2, NG, N], f32, tag="f2")
                    nc.tensor.matmul(f2_ps, fLR[:, r, 2:4], htaug[:, sl, :],
                                     start=True, stop=True)
                    nc.scalar.activation(out=f3b[:, sl, :], in_=f2_ps,
                                         func=AF.Copy)
                    # e[q,p] = f_dst[q] + f_src[p]  (rank-2 matmul per graph)
                    e_ps = patt.tile([128, NG, N], f32, tag="e", bufs=2)
                    for g in range(NG):
                        nc.tensor.matmul(e_ps[:, g, :], f3[:, ga + g, :],
                                         f3b[:, ga + g, :], start=True, stop=True)
                    # t = lrelu(e)  (ACT Prelu, in-place on PSUM)
                    nc.scalar.activation(out=e_ps, in_=e_ps, func=AF.Prelu,
                                         alpha=0.01)
                    # t += moff (0 / -25) via fp8 identity matmul accumulate
                    for g in range(NG):
                        nc.tensor.matmul(e_ps[:, g, :], eye8,
                                         mo8_sb[:, (ga + g) * N:(ga + g + 1) * N],
                                         start=False, stop=True,
                                         skip_group_check=True)
                    # P = exp(t) -> fp16 SBUF
                    nc.scalar.activation(out=P_sb[:, sl, :], in_=e_ps, func=AF.Exp)
                    # U = P^T @ [h|1] : messages + row-sums
                    u_ps = hu_ps[:, :, ATOM:2 * ATOM + 1]
                    for g in range(NG):
                        nc.tensor.matmul(u_ps[:, g, :], P_sb[:, ga + g, :],
                                         haug[:, ga + g, :], start=True, stop=True)
                    nc.vector.reciprocal(inv[:, sl], u_ps[:, :, ATOM])
                    i_b = inv[:, sl].unsqueeze(2).to_broadcast([128, NG, ATOM])
                    nc.vector.tensor_tensor(dlt[:, sl, :], u_ps[:, :, 0:ATOM],
                                            i_b, OP.mult)
                    # delta^T per graph, then state update on Pool
                    for g in range(NG):
                        nc.tensor.transpose(dT_ps[0:ATOM, ga + g, :],
                                            dlt[:, ga + g, :], i128)
                    nc.gpsimd.tensor_tensor(Snx[:, sl, :], S[:, sl, :],
                                            dT_ps[:, sl, :], OP.add)
                if dbg == r + 1:
                    xd = work.tile([ATOM, G, N], f32, tag="xd")
                    nc.vector.tensor_copy(xd, states[r + 1][0:ATOM])
                    nc.sync.dma_start(out=d_xdbg[:], in_=xd)
                if dbg in (4, 5, 6, 7, 8) and r == 0:
                    src = {4: htaug[0:ATOM], 5: P_sb, 6: f3,
                           7: haug, 8: dlt}[dbg]
                    xd = work.tile(_dbgshape, f32, tag="xd")
                    nc.vector.tensor_copy(xd, src)
                    nc.sync.dma_start(out=d_xdbg[:], in_=xd)

                # cell-branch of the decoder during round 0 (idle slots)
                if r == 0:
                    vc = singles.tile([128, 4, G], f16, tag="vc")
                    nc.scalar.activation(out=vc, in_=cellT_sb, func=AF.Exp,
                                         scale=-1.0)
                    nc.vector.tensor_scalar_add(vc, vc, 1.0)
                    nc.vector.reciprocal(vc, vc)
                    h1c_ps = deco_ps[:, 0:G]
                    for c in range(4):
                        nc.tensor.matmul(h1c_ps, w1[:, c + 1, :], vc[:, c, :],
                                         start=(c == 0), stop=(c == 3))
                    nc.vector.tensor_copy(h1c_sb, h1c_ps)

            # ---------------- VEC head + decoder --------------------------
            S3 = states[3]
            gts = singles.tile([128, 2, G, N], f16, tag="gts")
            d1_sb = singles.tile([ATOM, G, N], f16, tag="d1_sb")
            dm = singles.tile([128, G], f32, tag="dm")
            v0 = singles.tile([128, G], f16, tag="v0")
            for gi, sl in groups:
                # g = relu(Wt^T @ state3), two 128-halves
                for hh in range(2):
                    gt_ps = patt.tile([128, NG, N], f32, tag="e", bufs=2)
                    nc.tensor.matmul(gt_ps, wtaug[:, hh, :], S3[:, sl, :],
                                     start=True, stop=True)
                    if hh == 0:
                        nc.scalar.activation(out=gts[:, hh, sl, :], in_=gt_ps,
                                             func=AF.Relu)
                    else:
                        nc.vector.tensor_scalar_max(gts[:, hh, sl, :], gt_ps, 0.0)
                # d1 = Wf^T @ g + x0   (residual via identity matmul)
                d1_ps = pdT.tile([ATOM, NG, N], f32, tag="d1")
                for k in range(2):
                    nc.tensor.matmul(d1_ps, wf[:, k, :], gts[:, k, sl, :],
                                     start=(k == 0), stop=False)
                nc.tensor.matmul(d1_ps, i128[0:ATOM, 0:ATOM],
                                 state0[0:ATOM, sl, :], start=False, stop=True)
                nc.scalar.activation(out=d1_sb[:, sl, :], in_=d1_ps, func=AF.Copy)
                # d2 = Wf2^T @ d1 ; dm = max over nodes
                d2_ps = patt.tile([128, NG, N], f32, tag="e", bufs=2)
                nc.tensor.matmul(d2_ps, wf2, d1_sb[:, sl, :],
                                 start=True, stop=True)
                nc.vector.tensor_reduce(dm[:, sl], d2_ps, AX.X, OP.max)
                # v0 = sigmoid(dm + bias) = 1/(1+exp(-dm+b2n))
                nc.scalar.activation(out=v0[:, sl], in_=dm[:, sl], func=AF.Exp,
                                     bias=b2n, scale=-1.0)
                nc.vector.tensor_scalar_add(v0[:, sl], v0[:, sl], 1.0)
                nc.vector.reciprocal(v0[:, sl], v0[:, sl])
            if dbg == 9:
                xd = work.tile([128, 2, G, N], f32, tag="xd")
                nc.vector.tensor_copy(xd, gts)
                nc.sync.dma_start(out=d_xdbg[:], in_=xd)
            if dbg == 10:
                xd = work.tile([ATOM, G, N], f32, tag="xd")
                nc.vector.tensor_copy(xd, d1_sb)
                nc.sync.dma_start(out=d_xdbg[:], in_=xd)
            if dbg == 11:
                nc.sync.dma_start(out=d_xdbg[:], in_=dm)
            if dbg == 12:
                xd = work.tile([128, G], f32, tag="xd")
                nc.vector.tensor_copy(xd, v0)
                nc.sync.dma_start(out=d_xdbg[:], in_=xd)

            # h1 = relu(W1g^T v0 + h1c + b1)
            h1_ps = deco_ps[:, 8:8 + G]
            nc.tensor.matmul(h1_ps, w1[:, 0, :], v0, start=True, stop=True)
            h1 = work.tile([128, G], f16, tag="h1")
            nc.vector.scalar_tensor_tensor(h1, h1_ps, b1p, h1c_sb,
                                           OP.add, OP.add)
            nc.vector.tensor_scalar_max(h1, h1, 0.0)
            if dbg == 13:
                xd = work.tile([128, G], f32, tag="xd")
                nc.vector.tensor_copy(xd, h1)
                nc.sync.dma_start(out=d_xdbg[:], in_=xd)

            h2_ps = deco_ps[:, 16:32].rearrange("p (b g) -> p b g", b=2)
            for b in range(2):
                nc.tensor.matmul(h2_ps[:, b, :], w2[:, b, :], h1,
                                 start=True, stop=True)
            h2 = work.tile([128, 2, G], f16, tag="h2")
            for b in range(2):
                nc.vector.tensor_scalar(h2[:, b, :], h2_ps[:, b, :],
                                        b2d[:, b:b + 1], 0.0, OP.add, OP.max)
            if dbg == 14:
                xd = work.tile([128, 2, G], f32, tag="xd")
                nc.vector.tensor_copy(xd, h2)
                nc.sync.dma_start(out=d_xdbg[:], in_=xd)

            h3_ps = deco_ps[:, 32:64].rearrange("p (b g) -> p b g", b=4)
            for b in range(4):
                for kc in range(2):
                    nc.tensor.matmul(h3_ps[:, b, :], w3[:, kc, b, :],
                                     h2[:, kc, :], start=(kc == 0),
                                     stop=(kc == 1))
            h3 = work.tile([128, 4, G], f16, tag="h3")
            for b in range(4):
                nc.vector.tensor_scalar(h3[:, b, :], h3_ps[:, b, :],
                                        b3p[:, b:b + 1], 0.0, OP.add, OP.max)

            s_ps = deco_ps[0:1, 64:64 + G]
            for c in range(4):
                nc.tensor.matmul(s_ps, w4[:, c:c + 1], h3[:, c, :],
                                 start=(c == 0), stop=(c == 3))
            s_sb = work.tile([1, G], f32, tag="s_sb")
            nc.vector.tensor_scalar_add(s_sb, s_ps, b4p)
            if dbg == 15:
                nc.sync.dma_start(out=d_xdbg[:], in_=s_sb)
            nc.sync.dma_start(out=d_score.rearrange("g x -> x g"), in_=s_sb)

    lowp.__exit__(None, None, None)
    return nc


def _fix_preamble_regs(nc):
    """Bacc defers register allocation; its alloc_regs pass skips the
    framework preamble registers, leaving reg_id=-1 which walrus rejects.
    Assign collision-free ids."""
    per_engine_used = {}
    pending = []
    for alloc in nc.m.functions[0].allocations:
        eng = getattr(alloc, "engine", None)
        rid = getattr(alloc, "reg_id", None)
        if eng is None or rid is None:
            continue
        if rid >= 0:
            per_engine_used.setdefault(eng, set()).add(rid)
        else:
            pending.append(alloc)
    canonical = {"zero": 8, "monotonic_0_cnt": 9, "bcreg0_lo": 10,
                 "bcreg0_hi": 11, "bcreg1_lo": 12, "bcreg1_hi": 13,
                 "tpb_base_lo": 14, "tpb_base_hi": 15}
    for alloc in pending:
        eng = alloc.engine
        used = per_engine_used.setdefault(eng, set())
        suffix = alloc.name.split("_", 1)[1] if "_" in alloc.name else alloc.name
        rid = canonical.get(suffix, 16)
        while rid in used:
            rid += 1
        alloc.reg_id = rid
        used.add(rid)


def _stage(inputs):
    """Host-side layout staging (fp16/fp8 packing). Returns per-core in_maps."""
    import ml_dtypes

    f16 = np.float16
    xs = np.asarray(inputs["xs"], dtype=np.float32)
    A = np.asarray(inputs["A"])
    cell = np.asarray(inputs["cell_emb"], dtype=np.float32)
    Wg = np.asarray(inputs["Wg"], dtype=np.float32)
    bg = np.asarray(inputs["bg"], dtype=np.float32)
    attn = np.asarray(inputs["attn"], dtype=np.float32)
    Wt = np.asarray(inputs["Wt"], dtype=np.float32)
    bt = np.asarray(inputs["bt"], dtype=np.float32)
    Wf = np.asarray(inputs["Wf"], dtype=np.float32)
    bf = np.asarray(inputs["bf"], dtype=np.float32)
    Wf2 = np.asarray(inputs["Wf2"], dtype=np.float32)
    bf2 = np.asarray(inputs["bf2"], dtype=np.float32)
    W1 = np.asarray(inputs["W1"], dtype=np.float32)
    b1 = np.asarray(inputs["b1"], dtype=np.float32)
    W2 = np.asarray(inputs["W2"], dtype=np.float32)
    b2 = np.asarray(inputs["b2"], dtype=np.float32)
    W3 = np.asarray(inputs["W3"], dtype=np.float32)
    b3 = np.asarray(inputs["b3"], dtype=np.float32)
    W4 = np.asarray(inputs["W4"], dtype=np.float32)
    b4 = np.asarray(inputs["b4"], dtype=np.float32)

    FA = 3 * 35 + 12 + 128
    wA = np.zeros((128, FA), np.float32)
    wg4 = np.zeros((ATOM + 1, 3, ATOM + 1), np.float32)
    wg4[:ATOM, :, :ATOM] = Wg.transpose(1, 0, 2)
    wg4[ATOM, :, :ATOM] = bg
    wg4[ATOM, :, ATOM] = 1.0
    wA[:ATOM + 1, 0:105] = wg4.reshape(ATOM + 1, 105)
    fLR = np.zeros((ATOM + 1, 3, 4), np.float32)
    for r in range(3):
        fLR[:ATOM, r, 0] = attn[r, ATOM:]    # f_dst
        fLR[ATOM, r, 1] = 1.0                # ones
        fLR[ATOM, r, 2] = 1.0                # ones
        fLR[:ATOM, r, 3] = attn[r, :ATOM]    # f_src
    wA[:ATOM + 1, 105:117] = fLR.reshape(ATOM + 1, 12)
    wA[:, 117:] = np.eye(128, dtype=np.float32)

    FB = 256 + 68 + 128 + 640 + 256 + 1024 + 4
    wB = np.zeros((128, FB), np.float32)
    o = 0
    wB[:ATOM, o:o + 256] = Wt
    wB[ATOM, o:o + 256] = bt
    o += 256
    wB[:, o:o + 68] = Wf.reshape(2, 128, ATOM).transpose(1, 0, 2).reshape(128, 68)
    o += 68
    wB[:ATOM, o:o + 128] = Wf2
    o += 128
    wB[:, o:o + 640] = W1.reshape(5, 128, 128).transpose(1, 0, 2).reshape(128, 640)
    o += 640
    wB[:, o:o + 256] = W2.reshape(128, 2, 128).reshape(128, 256)
    o += 256
    wB[:, o:o + 1024] = W3.reshape(2, 128, 4, 128).transpose(1, 0, 2, 3).reshape(128, 1024)
    o += 1024
    wB[:, o:o + 4] = W4.reshape(4, 128).T
    o += 4

    b32 = np.zeros((128, 9), np.float32)
    b32[:, 0] = -(bf @ Wf2 + bf2)
    b32[:, 1] = b1
    b32[:, 2:4] = b2.reshape(2, 128).T
    b32[:, 4:8] = b3.reshape(4, 128).T
    b32[0, 8] = b4[0]

    shared = dict(wA=wA.astype(f16), wB=wB.astype(f16), b32=b32)

    in_maps = []
    for core in range(NCORES):
        sl = slice(core * G, (core + 1) * G)
        x0t = np.concatenate(
            [xs[sl].transpose(0, 2, 1),
             np.ones((G, 1, N), np.float32)], axis=1)      # [G, 35, N]
        x0t = np.ascontiguousarray(x0t.transpose(1, 0, 2)).astype(f16)
        # moff[q, g, p] = 0 if A[g, p, q] > 0 else MOFF ; plus eye cols
        m_qgp = (np.asarray(A[sl]) > 0).transpose(2, 0, 1)  # [q, g, p]
        mo = np.where(m_qgp, 0.0, MOFF).astype(np.float32).reshape(128, G * N)
        mo8 = np.concatenate([mo, np.eye(128, dtype=np.float32)], axis=1)
        cellT = np.ascontiguousarray(
            cell[sl].reshape(G, 4, 128).transpose(2, 1, 0)).astype(f16)
        m = dict(shared)
        m.update(x0t=x0t,
                 mo8=mo8.astype(ml_dtypes.float8_e4m3),
                 cellT=cellT)
        in_maps.append(m)
    return in_maps


def get_nc(dbg=0):
    key = f"nc{dbg}"
    if key not in _CACHE:
        nc = _build_bass(dbg)
        nc.finalize()
        _fix_preamble_regs(nc)
        _CACHE[key] = nc
    return _CACHE[key]


def kernel(**inputs) -> np.ndarray:
    from concourse.bass_utils import run_bass_kernel_spmd

    nc = get_nc()
    in_maps = _stage(inputs)
    res = run_bass_kernel_spmd(nc, in_maps, core_ids=list(range(NCORES)))
    out = np.concatenate([res.results[i]["score"] for i in range(NCORES)], axis=0)
    return out.astype(np.float32)


# revision 15
# speedup vs baseline: 2.1036x; 1.0237x over previous
"""Trainium2 Bass kernel for nn_DiseaseModel_mlp (GNN message passing + MLP decoder).

Data parallel over batch: 64 graphs -> 8 NeuronCores x 8 graphs; weights
replicated. Per core the 8 graphs are split into 2 groups of 4 that are
pipelined across engines (PE / ACT / DVE / Pool).

Key structure (fp16 on device, fp32 PSUM accumulation):
- state kept feature-major [35, G, 128] (row 34 = ones -> bias folding)
- per round: ht = relu(Wg^T @ state) (feature-major), h = relu(state^T Wg)
  (node-major), f = a12aug^T @ [ht;1] gives rows [f_dst, ones, f_src] in
  one matmul; e[q,p] = f_dst[q]+f_src[p] via one rank-2 matmul per graph;
  lrelu via ACT Prelu(alpha=.01) in-place on PSUM; mask folded as -25 on
  masked entries via an fp8 identity x moff matmul accumulated onto the
  same PSUM; exp on ACT; U = P^T @ [h|1] gives messages + softmax
  row-sums in one matmul. No DVE mask multiply, no max-subtraction.
- decoder cell_emb branch (4/5 of W1) precomputed during round 0.

Note: every matmul operand is kept at SBUF base partition 0 (operands at
other bases can hard-crash the device).
"""

import numpy as np

ATOM = 34
HID = 256
LATENT = 128
CELLS = 512
B, N = 64, 128
NCORES = 8
G = B // NCORES          # graphs per core = 8
NG = 4                   # graphs per pipeline group
MOFF = -25.0             # additive mask offset (post-lrelu), exp(-25+6)~0
N_WARM_MM = 12           # dummy matmuls to ramp the PE p-state during DMA

_CACHE = {}


def _build_bass(dbg=0):
    import concourse.bass as bass
    import concourse.bacc as bacc
    import concourse.mybir as mybir
    import concourse.tile as tile

    f32 = mybir.dt.float32
    f16 = mybir.dt.float16
    f8 = mybir.dt.float8e4
    AF = mybir.ActivationFunctionType
    OP = mybir.AluOpType
    AX = mybir.AxisListType

    nc = bacc.Bacc(None)

    FA = 3 * 35 + 12 + 128                      # wg4(105) fLR(12) i128(128)
    FB = 256 + 68 + 128 + 640 + 256 + 1024 + 4

    d_x0t = nc.declare_dram_parameter("x0t", [ATOM + 1, G, N], f16, isOutput=False)
    d_mo8 = nc.declare_dram_parameter("mo8", [128, G * N + 128], f8, isOutput=False)
    d_cellT = nc.declare_dram_parameter("cellT", [128, 4, G], f16, isOutput=False)
    d_wA = nc.declare_dram_parameter("wA", [128, FA], f16, isOutput=False)
    d_wB = nc.declare_dram_parameter("wB", [128, FB], f16, isOutput=False)
    d_b32 = nc.declare_dram_parameter("b32", [128, 9], f32, isOutput=False)
    d_score = nc.declare_dram_parameter("score", [G, 1], f32, isOutput=True)

    _dbgshape = {1: [ATOM, G, N], 2: [ATOM, G, N], 3: [ATOM, G, N],
                 4: [ATOM, G, N], 5: [128, G, N], 6: [2, G, N],
                 7: [128, G, ATOM + 1], 8: [128, G, ATOM], 9: [128, 2, G, N],
                 10: [ATOM, G, N], 11: [128, G], 12: [128, G],
                 13: [128, G], 14: [128, 2, G], 15: [1, G]}.get(dbg, [1, 1])
    d_xdbg = (nc.declare_dram_parameter("xdbg", _dbgshape, f32, isOutput=True)
              if dbg else None)

    groups = [(0, slice(0, NG)), (1, slice(NG, G))]

    lowp = nc.allow_low_precision(reason="fp16 pipeline; tolerance 2e-2")
    lowp.__enter__()
    with tile.TileContext(nc) as tc:
        with (
            tc.tile_pool(name="singles", bufs=1) as singles,
            tc.tile_pool(name="work", bufs=2) as work,
            tc.tile_pool(name="pbig", bufs=1, space="PSUM") as pbig,
            tc.tile_pool(name="patt", bufs=1, space="PSUM") as patt,
            tc.tile_pool(name="pmisc", bufs=1, space="PSUM") as pmisc,
            tc.tile_pool(name="pdT", bufs=1, space="PSUM") as pdT,
        ):
            # ---------------- input DMAs (issue ASAP, big ones first) ------
            state0 = singles.tile([ATOM + 1, G, N], f16, tag="state0")
            nc.sync.dma_start(out=state0[:, 0:NG, :], in_=d_x0t[:, 0:NG, :])
            wA_sb = singles.tile([128, FA], f16, tag="wA")
            nc.scalar.dma_start(out=wA_sb, in_=d_wA[:])
            nc.sync.dma_start(out=state0[:, NG:G, :], in_=d_x0t[:, NG:G, :])
            mo8_sb = singles.tile([128, G * N + 128], f8, tag="mo8")
            nc.scalar.dma_start(out=mo8_sb[:, 0:NG * N], in_=d_mo8[:, 0:NG * N])
            nc.sync.dma_start(out=mo8_sb[:, NG * N:], in_=d_mo8[:, NG * N:])
            b32_sb = singles.tile([128, 9], f32, tag="b32")
            nc.scalar.dma_start(out=b32_sb, in_=d_b32[:])
            cellT_sb = singles.tile([128, 4, G], f16, tag="cellT")
            nc.sync.dma_start(out=cellT_sb, in_=d_cellT[:])
            wB_sb = singles.tile([128, FB], f16, tag="wB")
            nc.scalar.dma_start(out=wB_sb, in_=d_wB[:])

            # weight views inside the blobs
            wg = wA_sb[0:ATOM + 1, 0:105].rearrange(
                "p (r d) -> p r d", r=3)
            fLR = wA_sb[0:ATOM + 1, 105:117].rearrange(
                "p (r c) -> p r c", r=3)
            i128 = wA_sb[:, 117:117 + 128]
            o = 0
            wtaug = wB_sb[0:ATOM + 1, o:o + 256].rearrange(
                "p (h m) -> p h m", h=2); o += 256
            wf = wB_sb[:, o:o + 68].rearrange("p (k d) -> p k d", k=2); o += 68
            wf2 = wB_sb[0:ATOM, o:o + 128]; o += 128
            w1 = wB_sb[:, o:o + 640].rearrange("p (k m) -> p k m", k=5); o += 640
            w2 = wB_sb[:, o:o + 256].rearrange("p (b m) -> p b m", b=2); o += 256
            w3 = wB_sb[:, o:o + 1024].rearrange(
                "p (k b m) -> p k b m", k=2, b=4); o += 1024
            w4 = wB_sb[:, o:o + 4]; o += 4
            b2n = b32_sb[:, 0:1]
            b1p = b32_sb[:, 1:2]
            b2d = b32_sb[:, 2:4]
            b3p = b32_sb[:, 4:8]
            b4p = b32_sb[0:1, 8:9]
            eye8 = mo8_sb[:, G * N:]

            # ---------------- static SBUF tiles ---------------------------
            # ACT table warm-up (Exp/Relu/Prelu/Copy share one table set)
            warm = singles.tile([1, 1], f32, tag="warm")
            nc.vector.memset(warm, 0.0)
            nc.scalar.activation(out=warm, in_=warm, func=AF.Exp)

            htaug = singles.tile([ATOM + 1, G, N], f16, tag="htaug")
            haug = singles.tile([128, G, ATOM + 1], f16, tag="haug")
            nc.vector.memset(haug[:, :, ATOM], 1.0)
            f3 = singles.tile([2, G, N], f16, tag="f3")
            f3b = singles.tile([2, G, N], f16, tag="f3b")
            P_sb = singles.tile([128, G, N], f16, tag="P_sb")
            inv = singles.tile([128, G], f32, tag="inv")
            dlt = singles.tile([128, G, ATOM], f16, tag="dlt")
            h1c_sb = singles.tile([128, G], f32, tag="h1c_sb")
            states = [state0]
            for r in range(3):
                st = singles.tile([ATOM + 1, G, N], f16, tag=f"state{r + 1}")
                states.append(st)
            # shared delta^T PSUM tile; row 34 stays 0 so the state update
            # can add all 35 rows (ones row survives: 1 + 0)
            dT_ps = pdT.tile([ATOM + 1, G, N], f16, tag="dT")
            nc.vector.memset(dT_ps.bitcast(f32), 0.0)

            # PE warm-up: dummy matmuls (no DMA deps) ramp the p-state
            zz = singles.tile([1, 128], f16, tag="zz")
            nc.vector.memset(zz, 0.0)
            misc_ps = pmisc.tile([128, 408], f32, tag="misc")
            hu_all = misc_ps[:, 0:280].rearrange("p (g c) -> p g c", g=NG)
            deco_ps = misc_ps[:, 280:408]
            wm_ps = deco_ps[0:1, 0:128]
            for i in range(N_WARM_MM):
                nc.tensor.matmul(wm_ps, zz[0:1, 0:1], zz, start=True, stop=True)

            # ---------------- GNN rounds ----------------------------------
            for r in range(3):
                S = states[r]
                Snx = states[r + 1]
                for gi, sl in groups:
                    ga = gi * NG
                    # ht = Wg^T @ state (feature-major h; row 34 = ones
                    # via the extra e34 column of wg4)
                    ht_ps = pbig.tile([ATOM + 1, NG, N], f32, tag="ht")
                    nc.tensor.matmul(ht_ps, wg[:, r, :], S[:, sl, :],
                                     start=True, stop=True)
                    # h node-major per graph
                    hu_ps = hu_all
                    h_ps = hu_ps[:, :, 0:ATOM]
                    for g in range(NG):
                        nc.tensor.matmul(h_ps[:, g, :], S[:, ga + g, :],
                                         wg[:, r, 0:ATOM], start=True, stop=True)
                    # evacuate with relu: ht -> htaug (ACT), h -> haug (DVE)
                    nc.scalar.activation(out=htaug[:, sl, :], in_=ht_ps,
                                         func=AF.Relu)
                    nc.vector.tensor_scalar_max(haug[:, sl, 0:ATOM], h_ps, 0.0)
                    # f matmuls: fL -> [f_dst; ones], fR -> [ones; f_src]
                    f_ps = pbig.tile([2, NG, N], f32, tag="f")
                    nc.tensor.matmul(f_ps, fLR[:, r, 0:2], htaug[:, sl, :],
                                     start=True, stop=True)
                    nc.vector.tensor_copy(f3[:, sl, :], f_ps)
                    f2_ps = pbig.tile([2, NG, N], f32, tag="f2")
                    nc.tensor.matmul(f2_ps, fLR[:, r, 2:4], htaug[:, sl, :],
                                     start=True, stop=True)
                    nc.scalar.activation(out=f3b[:, sl, :], in_=f2_ps,
                                         func=AF.Copy)
                    # e[q,p] = f_dst[q] + f_src[p]  (rank-2 matmul per graph)
                    e_ps = patt.tile([128, NG, N], f32, tag="e", bufs=2)
                    for g in range(NG):
                        nc.tensor.matmul(e_ps[:, g, :], f3[:, ga + g, :],
                                         f3b[:, ga + g, :], start=True, stop=True)
                    # t = lrelu(e)  (ACT Prelu, in-place on PSUM)
                    nc.scalar.activation(out=e_ps, in_=e_ps, func=AF.Prelu,
                                         alpha=0.01)
                    # t += moff (0 / -25) via fp8 identity matmul accumulate
                    for g in range(NG):
                        nc.tensor.matmul(e_ps[:, g, :], eye8,
                                         mo8_sb[:, (ga + g) * N:(ga + g + 1) * N],
                                         start=False, stop=True,
                                         skip_group_check=True)
                    # P = exp(t) -> fp16 SBUF
                    nc.scalar.activation(out=P_sb[:, sl, :], in_=e_ps, func=AF.Exp)
                    # U = P^T @ [h|1] : messages + row-sums
                    u_ps = hu_ps[:, :, ATOM:2 * ATOM + 1]
                    for g in range(NG):
                        nc.tensor.matmul(u_ps[:, g, :], P_sb[:, ga + g, :],
                                         haug[:, ga + g, :], start=True, stop=True)
                    nc.vector.reciprocal(inv[:, sl], u_ps[:, :, ATOM])
                    i_b = inv[:, sl].unsqueeze(2).to_broadcast([128, NG, ATOM])
                    nc.vector.tensor_tensor(dlt[:, sl, :], u_ps[:, :, 0:ATOM],
                                            i_b, OP.mult)
                    # delta^T per graph, then state update on Pool
                    for g in range(NG):
                        nc.tensor.transpose(dT_ps[0:ATOM, ga + g, :],
                                            dlt[:, ga + g, :], i128)
                    nc.vector.tensor_tensor(Snx[:, sl, :], S[:, sl, :],
                                            dT_ps[:, sl, :], OP.add)
                if dbg == r + 1:
                    xd = work.tile([ATOM, G, N], f32, tag="xd")
                    nc.vector.tensor_copy(xd, states[r + 1][0:ATOM])
                    nc.sync.dma_start(out=d_xdbg[:], in_=xd)
                if dbg in (4, 5, 6, 7, 8) and r == 0:
                    src = {4: htaug[0:ATOM], 5: P_sb, 6: f3,
                           7: haug, 8: dlt}[dbg]
                    xd = work.tile(_dbgshape, f32, tag="xd")
                    nc.vector.tensor_copy(xd, src)
                    nc.sync.dma_start(out=d_xdbg[:], in_=xd)

                # cell-branch of the decoder during round 0 (idle slots)
                if r == 0:
                    vc = singles.tile([128, 4, G], f16, tag="vc")
                    nc.scalar.activation(out=vc, in_=cellT_sb, func=AF.Exp,
                                         scale=-1.0)
                    nc.vector.tensor_scalar_add(vc, vc, 1.0)
                    nc.vector.reciprocal(vc, vc)
                    h1c_ps = deco_ps[:, 0:G]
                    for c in range(4):
                        nc.tensor.matmul(h1c_ps, w1[:, c + 1, :], vc[:, c, :],
                                         start=(c == 0), stop=(c == 3))
                    nc.vector.tensor_copy(h1c_sb, h1c_ps)

            # ---------------- VEC head + decoder --------------------------
            S3 = states[3]
            gts = singles.tile([128, 2, G, N], f16, tag="gts")
            d1_sb = singles.tile([ATOM, G, N], f16, tag="d1_sb")
            dm = singles.tile([128, G], f32, tag="dm")
            v0 = singles.tile([128, G], f16, tag="v0")
            for gi, sl in groups:
                # g = relu(Wt^T @ state3), two 128-halves
                for hh in range(2):
                    gt_ps = patt.tile([128, NG, N], f32, tag="e", bufs=2)
                    nc.tensor.matmul(gt_ps, wtaug[:, hh, :], S3[:, sl, :],
                                     start=True, stop=True)
                    if hh == 0:
                        nc.scalar.activation(out=gts[:, hh, sl, :], in_=gt_ps,
                                             func=AF.Relu)
                    else:
                        nc.vector.tensor_scalar_max(gts[:, hh, sl, :], gt_ps, 0.0)
                # d1 = Wf^T @ g + x0   (residual via identity matmul)
                d1_ps = pdT.tile([ATOM, NG, N], f32, tag="d1")
                for k in range(2):
                    nc.tensor.matmul(d1_ps, wf[:, k, :], gts[:, k, sl, :],
                                     start=(k == 0), stop=False)
                nc.tensor.matmul(d1_ps, i128[0:ATOM, 0:ATOM],
                                 state0[0:ATOM, sl, :], start=False, stop=True)
                nc.scalar.activation(out=d1_sb[:, sl, :], in_=d1_ps, func=AF.Copy)
                # d2 = Wf2^T @ d1 ; dm = max over nodes
                d2_ps = patt.tile([128, NG, N], f32, tag="e", bufs=2)
                nc.tensor.matmul(d2_ps, wf2, d1_sb[:, sl, :],
                                 start=True, stop=True)
                nc.vector.tensor_reduce(dm[:, sl], d2_ps, AX.X, OP.max)
                # v0 = sigmoid(dm + bias) = 1/(1+exp(-dm+b2n))
                nc.scalar.activation(out=v0[:, sl], in_=dm[:, sl], func=AF.Exp,
                                     bias=b2n, scale=-1.0)
                nc.vector.tensor_scalar_add(v0[:, sl], v0[:, sl], 1.0)
                nc.vector.reciprocal(v0[:, sl], v0[:, sl])
            if dbg == 9:
                xd = work.tile([128, 2, G, N], f32, tag="xd")
                nc.vector.tensor_copy(xd, gts)
                nc.sync.dma_start(out=d_xdbg[:], in_=xd)
            if dbg == 10:
                xd = work.tile([ATOM, G, N], f32, tag="xd")
                nc.vector.tensor_copy(xd, d1_sb)
                nc.sync.dma_start(out=d_xdbg[:], in_=xd)
            if dbg == 11:
                nc.sync.dma_start(out=d_xdbg[:], in_=dm)
            if dbg == 12:
                xd = work.tile([128, G], f32, tag="xd")
                nc.vector.tensor_copy(xd, v0)
                nc.sync.dma_start(out=d_xdbg[:], in_=xd)

            # h1 = relu(W1g^T v0 + h1c + b1)
            h1_ps = deco_ps[:, 8:8 + G]
            nc.tensor.matmul(h1_ps, w1[:, 0, :], v0, start=True, stop=True)
            h1 = work.tile([128, G], f16, tag="h1")
            nc.vector.scalar_tensor_tensor(h1, h1_ps, b1p, h1c_sb,
                                           OP.add, OP.add)
            nc.vector.tensor_scalar_max(h1, h1, 0.0)
            if dbg == 13:
                xd = work.tile([128, G], f32, tag="xd")
                nc.vector.tensor_copy(xd, h1)
                nc.sync.dma_start(out=d_xdbg[:], in_=xd)

            h2_ps = deco_ps[:, 16:32].rearrange("p (b g) -> p b g", b=2)
            for b in range(2):
                nc.tensor.matmul(h2_ps[:, b, :], w2[:, b, :], h1,
                                 start=True, stop=True)
            h2 = work.tile([128, 2, G], f16, tag="h2")
            for b in range(2):
                nc.vector.tensor_scalar(h2[:, b, :], h2_ps[:, b, :],
                                        b2d[:, b:b + 1], 0.0, OP.add, OP.max)
            if dbg == 14:
                xd = work.tile([128, 2, G], f32, tag="xd")
                nc.vector.tensor_copy(xd, h2)
                nc.sync.dma_start(out=d_xdbg[:], in_=xd)

            h3_ps = deco_ps[:, 32:64].rearrange("p (b g) -> p b g", b=4)
            for b in range(4):
                for kc in range(2):
                    nc.tensor.matmul(h3_ps[:, b, :], w3[:, kc, b, :],
                                     h2[:, kc, :], start=(kc == 0),
                                     stop=(kc == 1))
            h3 = work.tile([128, 4, G], f16, tag="h3")
            for b in range(4):
                nc.vector.tensor_scalar(h3[:, b, :], h3_ps[:, b, :],
                                        b3p[:, b:b + 1], 0.0, OP.add, OP.max)

            s_ps = deco_ps[0:1, 64:64 + G]
            for c in range(4):
                nc.tensor.matmul(s_ps, w4[:, c:c + 1], h3[:, c, :],
                                 start=(c == 0), stop=(c == 3))
            s_sb = work.tile([1, G], f32, tag="s_sb")
            nc.vector.tensor_scalar_add(s_sb, s_ps, b4p)
            if dbg == 15:
                nc.sync.dma_start(out=d_xdbg[:], in_=s_sb)
            nc.sync.dma_start(out=d_score.rearrange("g x -> x g"), in_=s_sb)

    lowp.__exit__(None, None, None)
    return nc


def _fix_preamble_regs(nc):
    """Bacc defers register allocation; its alloc_regs pass skips the
    framework preamble registers, leaving reg_id=-1 which walrus rejects.
    Assign collision-free ids."""
    per_engine_used = {}
    pending = []
    for alloc in nc.m.functions[0].allocations:
        eng = getattr(alloc, "engine", None)
        rid = getattr(alloc, "reg_id", None)
        if eng is None or rid is None:
            continue
        if rid >= 0:
            per_engine_used.setdefault(eng, set()).add(rid)
        else:
            pending.append(alloc)
    canonical = {"zero": 8, "monotonic_0_cnt": 9, "bcreg0_lo": 10,
                 "bcreg0_hi": 11, "bcreg1_lo": 12, "bcreg1_hi": 13,
                 "tpb_base_lo": 14, "tpb_base_hi": 15}
    for alloc in pending:
        eng = alloc.engine
        used = per_engine_used.setdefault(eng, set())
        suffix = alloc.name.split("_", 1)[1] if "_" in alloc.name else alloc.name
        rid = canonical.get(suffix, 16)
        while rid in used:
            rid += 1
        alloc.reg_id = rid
        used.add(rid)


def _stage(inputs):
    """Host-side layout staging (fp16/fp8 packing). Returns per-core in_maps."""
    import ml_dtypes

    f16 = np.float16
    xs = np.asarray(inputs["xs"], dtype=np.float32)
    A = np.asarray(inputs["A"])
    cell = np.asarray(inputs["cell_emb"], dtype=np.float32)
    Wg = np.asarray(inputs["Wg"], dtype=np.float32)
    bg = np.asarray(inputs["bg"], dtype=np.float32)
    attn = np.asarray(inputs["attn"], dtype=np.float32)
    Wt = np.asarray(inputs["Wt"], dtype=np.float32)
    bt = np.asarray(inputs["bt"], dtype=np.float32)
    Wf = np.asarray(inputs["Wf"], dtype=np.float32)
    bf = np.asarray(inputs["bf"], dtype=np.float32)
    Wf2 = np.asarray(inputs["Wf2"], dtype=np.float32)
    bf2 = np.asarray(inputs["bf2"], dtype=np.float32)
    W1 = np.asarray(inputs["W1"], dtype=np.float32)
    b1 = np.asarray(inputs["b1"], dtype=np.float32)
    W2 = np.asarray(inputs["W2"], dtype=np.float32)
    b2 = np.asarray(inputs["b2"], dtype=np.float32)
    W3 = np.asarray(inputs["W3"], dtype=np.float32)
    b3 = np.asarray(inputs["b3"], dtype=np.float32)
    W4 = np.asarray(inputs["W4"], dtype=np.float32)
    b4 = np.asarray(inputs["b4"], dtype=np.float32)

    FA = 3 * 35 + 12 + 128
    wA = np.zeros((128, FA), np.float32)
    wg4 = np.zeros((ATOM + 1, 3, ATOM + 1), np.float32)
    wg4[:ATOM, :, :ATOM] = Wg.transpose(1, 0, 2)
    wg4[ATOM, :, :ATOM] = bg
    wg4[ATOM, :, ATOM] = 1.0
    wA[:ATOM + 1, 0:105] = wg4.reshape(ATOM + 1, 105)
    fLR = np.zeros((ATOM + 1, 3, 4), np.float32)
    for r in range(3):
        fLR[:ATOM, r, 0] = attn[r, ATOM:]    # f_dst
        fLR[ATOM, r, 1] = 1.0                # ones
        fLR[ATOM, r, 2] = 1.0                # ones
        fLR[:ATOM, r, 3] = attn[r, :ATOM]    # f_src
    wA[:ATOM + 1, 105:117] = fLR.reshape(ATOM + 1, 12)
    wA[:, 117:] = np.eye(128, dtype=np.float32)

    FB = 256 + 68 + 128 + 640 + 256 + 1024 + 4
    wB = np.zeros((128, FB), np.float32)
    o = 0
    wB[:ATOM, o:o + 256] = Wt
    wB[ATOM, o:o + 256] = bt
    o += 256
    wB[:, o:o + 68] = Wf.reshape(2, 128, ATOM).transpose(1, 0, 2).reshape(128, 68)
    o += 68
    wB[:ATOM, o:o + 128] = Wf2
    o += 128
    wB[:, o:o + 640] = W1.reshape(5, 128, 128).transpose(1, 0, 2).reshape(128, 640)
    o += 640
    wB[:, o:o + 256] = W2.reshape(128, 2, 128).reshape(128, 256)
    o += 256
    wB[:, o:o + 1024] = W3.reshape(2, 128, 4, 128).transpose(1, 0, 2, 3).reshape(128, 1024)
    o += 1024
    wB[:, o:o + 4] = W4.reshape(4, 128).T
    o += 4

    b32 = np.zeros((128, 9), np.float32)
    b32[:, 0] = -(bf @ Wf2 + bf2)
    b32[:, 1] = b1
    b32[:, 2:4] = b2.reshape(2, 128).T
    b32[:, 4:8] = b3.reshape(4, 128).T
    b32[0, 8] = b4[0]

    shared = dict(wA=wA.astype(f16), wB=wB.astype(f16), b32=b32)

    in_maps = []
    for core in range(NCORES):
        sl = slice(core * G, (core + 1) * G)
        x0t = np.concatenate(
            [xs[sl].transpose(0, 2, 1),
             np.ones((G, 1, N), np.float32)], axis=1)      # [G, 35, N]
        x0t = np.ascontiguousarray(x0t.transpose(1, 0, 2)).astype(f16)
        # moff[q, g, p] = 0 if A[g, p, q] > 0 else MOFF ; plus eye cols
        m_qgp = (np.asarray(A[sl]) > 0).transpose(2, 0, 1)  # [q, g, p]
        mo = np.where(m_qgp, 0.0, MOFF).astype(np.float32).reshape(128, G * N)
        mo8 = np.concatenate([mo, np.eye(128, dtype=np.float32)], axis=1)
        cellT = np.ascontiguousarray(
            cell[sl].reshape(G, 4, 128).transpose(2, 1, 0)).astype(f16)
        m = dict(shared)
        m.update(x0t=x0t,
                 mo8=mo8.astype(ml_dtypes.float8_e4m3),
                 cellT=cellT)
        in_maps.append(m)
    return in_maps


def get_nc(dbg=0):
    key = f"nc{dbg}"
    if key not in _CACHE:
        nc = _build_bass(dbg)
        nc.finalize()
        _fix_preamble_regs(nc)
        _CACHE[key] = nc
    return _CACHE[key]


def kernel(**inputs) -> np.ndarray:
    from concourse.bass_utils import run_bass_kernel_spmd

    nc = get_nc()
    in_maps = _stage(inputs)
    res = run_bass_kernel_spmd(nc, in_maps, core_ids=list(range(NCORES)))
    out = np.concatenate([res.results[i]["score"] for i in range(NCORES)], axis=0)
    return out.astype(np.float32)
